# revision 13
# baseline (speedup 1.0000x reference)
"""CharCNN token embedder (ELMo-style) on 8 Trainium2 NeuronCores — v2.

Data-parallel over 4096 = 16*256 tokens (512 per core), weights replicated.

Per-core pipeline (all phases overlap via tile-framework semaphores):
  1. Chunked dma_gather (8 x 3584 idx) pulls char-embedding rows into
     feature-major chunks xg_r[d, (t', n)]; per-chunk shifted copies build
     the im2col patch matrix xs[(k,d), (t, n)] incrementally, so conv
     matmuls start while later chunks are still in flight.
  2. Conv = matmuls with K=112 per 128-channel tile, one per output
     position (rounds of <=4 positions into a [128,4,512] PSUM group,
     double-buffered).  Position max-pool runs as a statically scheduled
     mix of drain types balanced across engines:
       X: DVE folds PSUM directly into a bf16 acc pair
       Z: ACT copies PSUM->bf16, DVE folds
       W: ACT copies PSUM->bf16, GPSIMD(Pool) folds (Pool cannot read PSUM)
  3. Per-tile finale: combine accs, relu+bias -> h1 (bf16) + h8 (fp8e4).
  4. 2 highway layers: nonlinear half in bf16 (16 K-chunks), gate half in
     fp8e4 DoubleRow (8 chunk-pairs, 2x PE throughput; sigmoid gating makes
     gate-half quantization error negligible).  ACT does relu/sigmoid,
     DVE does the gating arithmetic.
  5. Projection to 512 in bf16; output stored feature-major [128, 4, 512]
     fp32 and transposed on the host.
"""

import numpy as np
import ml_dtypes

import concourse.bass as bass
import concourse.mybir as mybir
import concourse.tile as tile
from concourse import bacc
from concourse.bass_utils import run_bass_kernel_spmd
from concourse.vector_clock import ScopedClock

# ---------------------------------------------------------------- constants
B, S, L = 16, 256, 50
CHAR_DIM = 16
CHAR_VOCAB = 262
PAD_V = 264
ZERO_ROW = 262
FILTERS = [(1, 32), (2, 32), (3, 64), (4, 128), (5, 256), (6, 512), (7, 1024)]
N_FILTERS = 2048
PROJ_DIM = 512
N_CORES = 8
NTOK = B * S
TOK = NTOK // N_CORES        # 512 tokens per core
TP = 56                      # padded positions per token (50 + 6)
NI = TOK * TP                # gather indices per core = 28672
# gather chunk boundaries in t'-positions: small head chunk so conv starts
# early, small tail so xg tiles stay <= 3584 indices
CHUNK_BOUNDS = [(0, 8), (8, 15), (15, 22), (22, 29),
                (29, 36), (36, 43), (43, 50), (50, 56)]
NPOS = 50
FREE = TOK * NPOS            # 25600
KDIM = 112

# per-tile valid position count (tile 0 = w1/w2/w3 mixed; see tail handling)
TILE_TCNT = [50, 47, 46, 46, 45, 45, 45, 45, 44, 44, 44, 44, 44, 44, 44, 44]

BF16 = mybir.dt.bfloat16
FP32 = mybir.dt.float32
FP8 = mybir.dt.float8e4

_MAX_WAITS_PER_INST = 1


def _patched_drain_and_barrier(self, tick_clock, wait_clock):
    # The walrus build in this container rejects CTRL instructions carrying
    # more than one sem wait; spread the kernel-tail drain waits over NOPs.
    nc = self.nc
    carrier = nc.sync.nop()
    wait_clock.add_sem_waits(carrier.ins, ScopedClock({None: tick_clock.global_clock}))
    si = carrier.ins.sync_info
    waits = list(si.on_wait) if si is not None and si.on_wait else []
    if len(waits) > _MAX_WAITS_PER_INST:
        carrier.ins.sync_info = mybir.SyncInfo(
            on_wait=waits[:_MAX_WAITS_PER_INST],
            on_update=list(si.on_update) if si.on_update else [])
        for i in range(_MAX_WAITS_PER_INST, len(waits), _MAX_WAITS_PER_INST):
            extra = nc.sync.nop()
            extra.ins.sync_info = mybir.SyncInfo(
                on_wait=waits[i:i + _MAX_WAITS_PER_INST], on_update=[])
    nc.sync.drain()
    nc.all_engine_barrier()
    assert self.sems is not None
    popped = nc._tile_sem_poison_stack.pop()
    assert popped is self._sem_poison
    nc.clear_and_free_semaphores(list(self.sems.allocated().values()))
    nc.all_engine_barrier()


tile.TileContext._drain_and_barrier = _patched_drain_and_barrier


# ------------------------------------------------------- static drain plan
def build_conv_schedule():
    """Greedy X/Z assignment of conv pooling rounds (Pool engine cannot run
    elementwise ops on TRN2, so only DVE folds PSUM directly (X) or folds
    ACT-staged bf16 copies (Z)).  Finale relu+bias runs on DVE via 4x-mode
    tensor_scalar; h8 casts ride on gpsimd cast-DMAs."""
    cost = {
        "X": {2: [("DVE", 1192)], 1: [("DVE", 658)]},
        "Z": {2: [("ACT", 1038), ("DVE", 594)], 1: [("ACT", 612), ("DVE", 387)]},
    }
    load = {"DVE": 9000.0, "ACT": 9000.0}
    fold_eng = {"X": "DVE", "Z": "DVE"}
    cursors = [0] * 16
    order = []
    while True:
        alive = [i for i in range(16) if cursors[i] < TILE_TCNT[i]]
        if not alive:
            break
        alive.sort(key=lambda i: -(TILE_TCNT[i] - cursors[i]))
        for i in alive:
            tcnt = TILE_TCNT[i]
            t0 = cursors[i]
            if i == 0 and t0 == 48:
                order.append((i, t0, 2, "T0TAIL", False))
                load["DVE"] += 1316
                cursors[i] = tcnt
            else:
                lim = 48 if i == 0 else tcnt
                nt = min(2, lim - t0)
                if t0 == 0:
                    # tile init: direct copy into the acc (no fold) on the
                    # less-loaded PSUM-capable engine
                    if load["ACT"] + 1038 < load["DVE"] + 1192:
                        order.append((i, t0, nt, "Z0", True))
                        load["ACT"] += 1038
                    else:
                        order.append((i, t0, nt, "X", True))
                        load["DVE"] += 1192
                    cursors[i] = nt
                    continue
                best, bestkey = None, None
                for typ in ("X", "Z"):
                    m = max(max(load.values()),
                            *[load[e] + c for e, c in cost[typ][nt]])
                    key = (m, load["DVE"] + dict(cost[typ][nt]).get("DVE", 0.0))
                    if bestkey is None or key < bestkey:
                        best, bestkey = typ, key
                order.append((i, t0, nt, best, t0 == 0))
                for e, c in cost[best][nt]:
                    load[e] += c
                cursors[i] = t0 + nt
            if cursors[i] >= TILE_TCNT[i]:
                load["DVE"] += 520  # m1 max + tensor_scalar relu+bias
    return order, load


CONV_SCHED, CONV_LOAD = build_conv_schedule()

def conv_completion_order(sched):
    seen = []
    for (i, t0, nt, typ, first) in sched:
        done = (t0 + nt >= TILE_TCNT[i]) or typ == "T0TAIL"
        if done:
            seen.append(i)
    return seen

C_ORDER = conv_completion_order(CONV_SCHED)



# ---------------------------------------------------------------- device IR
def build_module():
    nc = bacc.Bacc()
    SIdx = NI // 16  # 1792 int16 columns

    table = nc.dram_tensor("table", [PAD_V, 128], BF16, kind="ExternalInput")
    idx = nc.dram_tensor("idx", [128, SIdx], mybir.dt.int16, kind="ExternalInput")
    wconv = nc.dram_tensor("wconv", [KDIM, N_FILTERS], BF16, kind="ExternalInput")
    bconv = nc.dram_tensor("bconv", [128, 16], FP32, kind="ExternalInput")
    whw_nl = nc.dram_tensor("whw_nl", [2, 16, 128, 10, 128], BF16, kind="ExternalInput")
    whw_n8 = nc.dram_tensor("whw_n8", [2, 16, 128, 3, 2, 128], FP8, kind="ExternalInput")
    whw_g8 = nc.dram_tensor("whw_g8", [2, 16, 128, 8, 2, 128], FP8, kind="ExternalInput")
    bhw = nc.dram_tensor("bhw", [128, 2, 16, 2], FP32, kind="ExternalInput")
    wproj = nc.dram_tensor("wproj", [4, 128, 16, 128], BF16, kind="ExternalInput")
    bproj = nc.dram_tensor("bproj", [128, 4], FP32, kind="ExternalInput")
    out = nc.dram_tensor("out", [128, 4, TOK], FP32, kind="ExternalOutput")

    with tile.TileContext(nc) as tc:
        with (
            tc.tile_pool(name="consts", bufs=1) as cpool,
            tc.tile_pool(name="gather", bufs=2) as gpool,
            tc.tile_pool(name="xs", bufs=1) as xspool,
            tc.tile_pool(name="accs", bufs=1) as apool,
            tc.tile_pool(name="stage", bufs=4) as spool,
            tc.tile_pool(name="hbuf", bufs=2) as hpool,
            tc.tile_pool(name="wstream", bufs=2) as wpool,
            tc.tile_pool(name="small", bufs=2) as mpool,
            tc.tile_pool(name="convp", bufs=4, space="PSUM") as convp,
        ):
            # ---- constants in
            idx_t = cpool.tile([128, SIdx], mybir.dt.int16)
            nc.sync.dma_start(out=idx_t[:], in_=idx[:])
            wconv_t = cpool.tile([KDIM, N_FILTERS], BF16)
            nc.sync.dma_start(out=wconv_t[:], in_=wconv[:])
            bconv_t = cpool.tile([128, 16], FP32)
            nc.sync.dma_start(out=bconv_t[:], in_=bconv[:])
            bhw_t = cpool.tile([128, 2, 16, 2], FP32)
            nc.sync.dma_start(out=bhw_t[:], in_=bhw[:])
            bproj_t = cpool.tile([128, 4], FP32)
            nc.sync.dma_start(out=bproj_t[:], in_=bproj[:])

            # ---- 1. chunked gather + incremental im2col build
            xs = xspool.tile([KDIM, FREE], BF16)
            for (p_lo, p_hi) in CHUNK_BOUNDS:
                n_idx = (p_hi - p_lo) * TOK
                xgc = gpool.tile([128, 1, 4096], BF16, tag="xg")
                nc.gpsimd.dma_gather(
                    out_ap=xgc[:, :, 0:n_idx],
                    in_ap=table[:],
                    idxs_ap=idx_t[:, p_lo * 32:p_hi * 32],
                    num_idxs=n_idx,
                    num_idxs_reg=n_idx,
                    elem_size=128,
                    transpose=True,
                    single_packet=False,
                )
                piece_eng = ([nc.sync, nc.scalar] if p_lo == 0
                             else [nc.sync])
                for k in range(7):
                    t_lo = max(0, p_lo - k)
                    t_hi = min(NPOS, p_hi - k)
                    if t_lo >= t_hi:
                        continue
                    s0 = t_lo + k - p_lo
                    piece_eng[k % len(piece_eng)].dma_start(
                        out=xs[16 * k:16 * (k + 1), TOK * t_lo:TOK * t_hi],
                        in_=xgc[0:16, 0, TOK * s0:TOK * (s0 + (t_hi - t_lo))],
                    )

            # ---- 2+3. conv rounds (static schedule) + finales
            h1 = hpool.tile([128, 16, TOK], BF16, tag="h")
            h8a = hpool.tile([128, 16, TOK], FP8, tag="h8")
            accd = {}
            accp = {}
            accp_used = set()
            n_done = 0
            h8_on_dve = {}
            for (i, t0, nt, typ, first) in CONV_SCHED:
                lhsT = wconv_t[:, 128 * i:128 * (i + 1)]
                P = convp.tile([128, 2, TOK], FP32, tag="ps")
                for rpos in range(nt):
                    t = t0 + rpos
                    nc.tensor.matmul(
                        out=P[:, rpos, :], lhsT=lhsT,
                        rhs=xs[:, TOK * t:TOK * (t + 1)],
                        start=True, stop=True,
                    )
                if first:
                    accd[i] = apool.tile([128, 2, TOK], BF16, tag=f"accd{i}", name=f"accd{i}")
                ad = accd[i]
                if typ == "Z0":
                    nc.scalar.copy(out=ad[:, 0:nt, :], in_=P[:, 0:nt, :])
                elif typ == "T0TAIL":
                    # pos t0 valid rows 0:64 (w1+w2), pos t0+1 rows 0:32 (w1)
                    nc.vector.tensor_tensor(
                        out=ad[0:64, 0:1, :], in0=ad[0:64, 0:1, :],
                        in1=P[0:64, 0:1, :], op=mybir.AluOpType.max)
                    nc.vector.tensor_tensor(
                        out=ad[0:32, 1:2, :], in0=ad[0:32, 1:2, :],
                        in1=P[0:32, 1:2, :], op=mybir.AluOpType.max)
                elif typ == "X":
                    if first:
                        nc.vector.tensor_scalar_add(
                            out=ad[:, 0:nt, :], in0=P[:, 0:nt, :], scalar1=0.0)
                    else:
                        nc.vector.tensor_tensor(
                            out=ad[:, 0:nt, :], in0=ad[:, 0:nt, :],
                            in1=P[:, 0:nt, :], op=mybir.AluOpType.max)
                else:
                    s4 = spool.tile([128, 2, TOK], BF16, tag="s4")
                    nc.scalar.copy(out=s4[:, 0:nt, :], in_=P[:, 0:nt, :])
                    if typ == "Z":
                        if first:
                            nc.vector.tensor_scalar_add(
                                out=ad[:, 0:nt, :], in0=s4[:, 0:nt, :], scalar1=0.0)
                        else:
                            nc.vector.tensor_tensor(
                                out=ad[:, 0:nt, :], in0=ad[:, 0:nt, :],
                                in1=s4[:, 0:nt, :], op=mybir.AluOpType.max)
                    else:  # W: pool folds
                        if i not in accp:
                            accp[i] = apool.tile([128, 2, TOK], BF16, tag=f"accp{i}", name=f"accp{i}")
                            nc.gpsimd.tensor_scalar_add(
                                out=accp[i][:, 0:nt, :], in0=s4[:, 0:nt, :], scalar1=0.0)
                        else:
                            ap_ = accp[i]
                            nc.gpsimd.scalar_tensor_tensor(
                                out=ap_[:, 0:nt, :], in0=s4[:, 0:nt, :], scalar=0.0,
                                in1=ap_[:, 0:nt, :],
                                op0=mybir.AluOpType.add, op1=mybir.AluOpType.max)
                        accp_used.add(i)
                # finale when tile complete
                done = (t0 + nt >= TILE_TCNT[i]) or (typ == "T0TAIL")
                if done:
                    if i in accp:
                        nc.vector.tensor_tensor(
                            out=ad[:], in0=ad[:], in1=accp[i][:],
                            op=mybir.AluOpType.max)
                    m1 = mpool.tile([128, TOK], BF16, tag="m1")
                    nc.vector.tensor_tensor(
                        out=m1[:], in0=ad[:, 0, :], in1=ad[:, 1, :],
                        op=mybir.AluOpType.max)
                    nc.vector.tensor_scalar(
                        out=h1[:, i, :], in0=m1[:],
                        scalar1=bconv_t[:, i:i + 1], scalar2=0.0,
                        op0=mybir.AluOpType.add, op1=mybir.AluOpType.max)
                    nc.gpsimd.dma_start(out=h8a[:, i, :], in_=h1[:, i, :])
                    n_done += 1

            # ---- 4. highway layers
            h_in, h8_in = h1, h8a
            for layer in range(2):
                h_out = hpool.tile([128, 16, TOK], BF16, tag="h", name=f"h_l{layer}")
                h8_out = None
                if layer == 0:
                    h8_out = hpool.tile([128, 16, TOK], FP8, tag="h8", name="h8b")
                for j in range(16):
                    wn = wpool.tile([128, 10, 128], BF16, tag="wnh", bufs=3)
                    nc.sync.dma_start(out=wn[:], in_=whw_nl[layer, j])
                    wn8 = wpool.tile([128, 3, 2, 128], FP8, tag="wn8", bufs=3)
                    nc.sync.dma_start(out=wn8[:], in_=whw_n8[layer, j])
                    wg = wpool.tile([128, 8, 2, 128], FP8, tag="wg")
                    nc.sync.dma_start(out=wg[:], in_=whw_g8[layer, j])
                    p_nl = convp.tile([128, 2, TOK], FP32, tag="ps", name="psnl")[:, 0, :]
                    corder = [c for c in (C_ORDER if layer == 0 else range(16)) if c < 10]
                    for ci, c in enumerate(corder):
                        nc.tensor.matmul(
                            out=p_nl[:], lhsT=wn[:, c, :], rhs=h_in[:, c, :],
                            start=(ci == 0), stop=False)
                    for c2 in (5, 6, 7):
                        nc.tensor.matmul(
                            out=p_nl[:], lhsT=wn8[:, c2 - 5],
                            rhs=h8_in[:, 2 * c2:2 * c2 + 2, :],
                            start=False, stop=(c2 == 7),
                            perf_mode=mybir.MatmulPerfMode.DoubleRow)
                    p_g = convp.tile([128, 2, TOK], FP32, tag="ps", name="psg")[:, 0, :]
                    for c2 in range(8):
                        nc.tensor.matmul(
                            out=p_g[:], lhsT=wg[:, c2], rhs=h8_in[:, 2 * c2:2 * c2 + 2, :],
                            start=(c2 == 0), stop=(c2 == 7),
                            perf_mode=mybir.MatmulPerfMode.DoubleRow)
                    nl = mpool.tile([128, TOK], BF16, tag="nl")
                    gt = mpool.tile([128, TOK], BF16, tag="gt")
                    nc.scalar.activation(
                        out=nl[:], in_=p_nl[:],
                        func=mybir.ActivationFunctionType.Relu,
                        bias=bhw_t[:, layer, j, 0:1], scale=1.0)
                    nc.scalar.activation(
                        out=gt[:], in_=p_g[:],
                        func=mybir.ActivationFunctionType.Sigmoid,
                        bias=bhw_t[:, layer, j, 1:2], scale=1.0)
                    d = mpool.tile([128, TOK], BF16, tag="d")
                    nc.vector.tensor_tensor(
                        out=d[:], in0=h_in[:, j, :], in1=nl[:],
                        op=mybir.AluOpType.subtract)
                    nc.vector.tensor_mul(out=gt[:], in0=gt[:], in1=d[:])
                    nc.vector.tensor_add(out=h_out[:, j, :], in0=nl[:], in1=gt[:])
                    if h8_out is not None:
                        nc.gpsimd.dma_start(out=h8_out[:, j, :], in_=h_out[:, j, :])
                h_in = h_out
                if h8_out is not None:
                    h8_in = h8_out

            # ---- 5. projection, feature-major out
            for j2 in range(4):
                wp = wpool.tile([128, 16, 128], BF16, tag="wn", bufs=2)
                nc.sync.dma_start(out=wp[:], in_=wproj[j2])
                p_o = convp.tile([128, 2, TOK], FP32, tag="ps", name="pso")[:, 0, :]
                for c in range(16):
                    nc.tensor.matmul(
                        out=p_o[:], lhsT=wp[:, c, :], rhs=h_in[:, c, :],
                        start=(c == 0), stop=(c == 15))
                ot = cpool.tile([128, TOK], FP32, tag="ot", name="ot", bufs=2)
                nc.scalar.activation(
                    out=ot[:], in_=p_o[:],
                    func=mybir.ActivationFunctionType.Identity,
                    bias=bproj_t[:, j2:j2 + 1], scale=1.0)
                nc.sync.dma_start(out=out[:, j2, :], in_=ot[:])

    nc.compile()
    return nc


_CACHED = {}


def _prep(inputs):
    """Host-side layout prep: sharding, index packing, weight packing."""
    chars = np.asarray(inputs["chars"]).astype(np.int64).reshape(NTOK, L)
    chars_pad = np.full((NTOK, TP), ZERO_ROW, np.int64)
    chars_pad[:, :L] = chars

    emb = np.asarray(inputs["char_emb"], np.float32)
    table = np.zeros((PAD_V, 128), np.float32)
    table[:CHAR_VOCAB, :CHAR_DIM] = emb
    table = table.astype(ml_dtypes.bfloat16)

    wc = np.zeros((7, CHAR_DIM, N_FILTERS), np.float32)
    off = 0
    for fi, (w, n) in enumerate(FILTERS):
        cw = np.asarray(inputs[f"conv_w_{fi}"], np.float32)  # (n, 16, w)
        wc[:w, :, off:off + n] = cw.transpose(2, 1, 0)
        off += n
    wconv = wc.reshape(KDIM, N_FILTERS).astype(ml_dtypes.bfloat16)
    bconv = np.concatenate([np.asarray(inputs[f"conv_b_{i}"], np.float32)
                            for i in range(7)])
    bconv_dev = bconv.reshape(16, 128).T.copy()  # (128, 16)

    whw_nl = np.zeros((2, 16, 128, 10, 128), np.float32)
    whw_n8 = np.zeros((2, 16, 128, 3, 2, 128), np.float32)
    whw_g8 = np.zeros((2, 16, 128, 8, 2, 128), np.float32)
    bhw = np.zeros((128, 2, 16, 2), np.float32)
    for l in range(2):
        W = np.asarray(inputs[f"hw_w_{l}"], np.float32)   # (4096, 2048)
        bb = np.asarray(inputs[f"hw_b_{l}"], np.float32)  # (4096,)
        WT = W.T  # (ic 2048, oc 4096)
        # nl: (j, p, c, o) = WT[128c+p, 128j+o]; chunks 12-15 go fp8 DR
        nlv = WT[:, 0:2048].reshape(16, 128, 16, 128)       # (c, p, j, o)
        whw_nl[l] = nlv[0:10].transpose(2, 1, 0, 3)         # (j, p, c, o)
        n8v = nlv[10:16].reshape(3, 2, 128, 16, 128)        # (c2, i, p, j, o)
        whw_n8[l] = n8v.transpose(3, 2, 0, 1, 4)            # (j, p, c2, i, o)
        gv = WT[:, 2048:4096].reshape(8, 2, 128, 16, 128)   # (c2, i, p, j, o)
        whw_g8[l] = gv.transpose(3, 2, 0, 1, 4)             # (j, p, c2, i, o)
        bhw[:, l, :, 0] = bb[0:2048].reshape(16, 128).T
        bhw[:, l, :, 1] = bb[2048:4096].reshape(16, 128).T
    whw_nl = whw_nl.astype(ml_dtypes.bfloat16)
    whw_n8 = whw_n8.astype(ml_dtypes.float8_e4m3)
    whw_g8 = whw_g8.astype(ml_dtypes.float8_e4m3)

    Wp = np.asarray(inputs["proj_w"], np.float32)  # (512, 2048)
    WpT = Wp.T  # (2048, 512)
    wproj = WpT.reshape(16, 128, 4, 128).transpose(2, 1, 0, 3).copy()
    wproj = wproj.astype(ml_dtypes.bfloat16)      # (j2, p, c, o)
    bproj = np.asarray(inputs["proj_b"], np.float32).reshape(4, 128).T.copy()

    shared = dict(table=table, wconv=wconv, bconv=bconv_dev, whw_nl=whw_nl, whw_n8=whw_n8,
                  whw_g8=whw_g8, bhw=bhw, wproj=wproj, bproj=bproj)

    in_maps = []
    for core in range(N_CORES):
        cp = chars_pad[core * TOK:(core + 1) * TOK]  # (512, 56)
        idx_flat = cp.T.reshape(-1).astype(np.int16)  # j = t'*512 + n
        idx16 = idx_flat.reshape(NI // 16, 16).T.copy()  # (16, NI/16)
        idx16 = np.tile(idx16, (8, 1))  # (128, NI/16)
        m = dict(shared)
        m["idx"] = idx16
        in_maps.append(m)
    return in_maps


def kernel(**inputs) -> np.ndarray:
    if "nc" not in _CACHED:
        _CACHED["nc"] = build_module()
    nc = _CACHED["nc"]
    in_maps = _prep(inputs)
    res = run_bass_kernel_spmd(nc, in_maps, core_ids=list(range(N_CORES)))
    # out[p, j2, n] -> full[n, 128*j2 + p]
    parts = []
    for r in res.results:
        o = r["out"]  # (128, 4, 512)
        parts.append(o.transpose(2, 1, 0).reshape(TOK, PROJ_DIM))
    full = np.concatenate(parts, axis=0)
    return full.reshape(B, S, PROJ_DIM)


if __name__ == "__main__":
    order, load = CONV_SCHED, CONV_LOAD
    from collections import Counter
    print("rounds:", len(order), Counter(t for (_, _, _, t, _) in order))
    print("loads (us):", {k: round(v / 1000, 1) for k, v in load.items()})


# revision 14
# speedup vs baseline: 1.0084x; 1.0084x over previous
"""CharCNN token embedder (ELMo-style) on 8 Trainium2 NeuronCores — v2.

Data-parallel over 4096 = 16*256 tokens (512 per core), weights replicated.

Per-core pipeline (all phases overlap via tile-framework semaphores):
  1. Chunked dma_gather (8 x 3584 idx) pulls char-embedding rows into
     feature-major chunks xg_r[d, (t', n)]; per-chunk shifted copies build
     the im2col patch matrix xs[(k,d), (t, n)] incrementally, so conv
     matmuls start while later chunks are still in flight.
  2. Conv = matmuls with K=112 per 128-channel tile, one per output
     position (rounds of <=4 positions into a [128,4,512] PSUM group,
     double-buffered).  Position max-pool runs as a statically scheduled
     mix of drain types balanced across engines:
       X: DVE folds PSUM directly into a bf16 acc pair
       Z: ACT copies PSUM->bf16, DVE folds
       W: ACT copies PSUM->bf16, GPSIMD(Pool) folds (Pool cannot read PSUM)
  3. Per-tile finale: combine accs, relu+bias -> h1 (bf16) + h8 (fp8e4).
  4. 2 highway layers: nonlinear half in bf16 (16 K-chunks), gate half in
     fp8e4 DoubleRow (8 chunk-pairs, 2x PE throughput; sigmoid gating makes
     gate-half quantization error negligible).  ACT does relu/sigmoid,
     DVE does the gating arithmetic.
  5. Projection to 512 in bf16; output stored feature-major [128, 4, 512]
     fp32 and transposed on the host.
"""

import numpy as np
import ml_dtypes

import concourse.bass as bass
import concourse.mybir as mybir
import concourse.tile as tile
from concourse import bacc
from concourse.bass_utils import run_bass_kernel_spmd
from concourse.vector_clock import ScopedClock

# ---------------------------------------------------------------- constants
B, S, L = 16, 256, 50
CHAR_DIM = 16
CHAR_VOCAB = 262
PAD_V = 264
ZERO_ROW = 262
FILTERS = [(1, 32), (2, 32), (3, 64), (4, 128), (5, 256), (6, 512), (7, 1024)]
N_FILTERS = 2048
PROJ_DIM = 512
N_CORES = 8
NTOK = B * S
TOK = NTOK // N_CORES        # 512 tokens per core
TP = 56                      # padded positions per token (50 + 6)
NI = TOK * TP                # gather indices per core = 28672
# gather chunk boundaries in t'-positions: small head chunk so conv starts
# early, small tail so xg tiles stay <= 3584 indices
CHUNK_BOUNDS = [(0, 8), (8, 15), (15, 22), (22, 29),
                (29, 36), (36, 43), (43, 50), (50, 56)]
NPOS = 50
FREE = TOK * NPOS            # 25600
KDIM = 112

# per-tile valid position count (tile 0 = w1/w2/w3 mixed; see tail handling)
TILE_TCNT = [50, 47, 46, 46, 45, 45, 45, 45, 44, 44, 44, 44, 44, 44, 44, 44]

BF16 = mybir.dt.bfloat16
FP32 = mybir.dt.float32
FP8 = mybir.dt.float8e4

_MAX_WAITS_PER_INST = 1


def _patched_drain_and_barrier(self, tick_clock, wait_clock):
    # The walrus build in this container rejects CTRL instructions carrying
    # more than one sem wait; spread the kernel-tail drain waits over NOPs.
    nc = self.nc
    carrier = nc.sync.nop()
    wait_clock.add_sem_waits(carrier.ins, ScopedClock({None: tick_clock.global_clock}))
    si = carrier.ins.sync_info
    waits = list(si.on_wait) if si is not None and si.on_wait else []
    if len(waits) > _MAX_WAITS_PER_INST:
        carrier.ins.sync_info = mybir.SyncInfo(
            on_wait=waits[:_MAX_WAITS_PER_INST],
            on_update=list(si.on_update) if si.on_update else [])
        for i in range(_MAX_WAITS_PER_INST, len(waits), _MAX_WAITS_PER_INST):
            extra = nc.sync.nop()
            extra.ins.sync_info = mybir.SyncInfo(
                on_wait=waits[i:i + _MAX_WAITS_PER_INST], on_update=[])
    nc.sync.drain()
    nc.all_engine_barrier()
    assert self.sems is not None
    popped = nc._tile_sem_poison_stack.pop()
    assert popped is self._sem_poison
    nc.clear_and_free_semaphores(list(self.sems.allocated().values()))
    nc.all_engine_barrier()


tile.TileContext._drain_and_barrier = _patched_drain_and_barrier


# ------------------------------------------------------- static drain plan
def build_conv_schedule():
    """Greedy X/Z assignment of conv pooling rounds (Pool engine cannot run
    elementwise ops on TRN2, so only DVE folds PSUM directly (X) or folds
    ACT-staged bf16 copies (Z)).  Finale relu+bias runs on DVE via 4x-mode
    tensor_scalar; h8 casts ride on gpsimd cast-DMAs."""
    cost = {
        "X": {2: [("DVE", 1192)], 1: [("DVE", 658)]},
        "Z": {2: [("ACT", 1038), ("DVE", 594)], 1: [("ACT", 612), ("DVE", 387)]},
    }
    load = {"DVE": 9000.0, "ACT": 9000.0}
    fold_eng = {"X": "DVE", "Z": "DVE"}
    cursors = [0] * 16
    order = []
    while True:
        alive = [i for i in range(16) if cursors[i] < TILE_TCNT[i]]
        if not alive:
            break
        alive.sort(key=lambda i: -(TILE_TCNT[i] - cursors[i]))
        for i in alive:
            tcnt = TILE_TCNT[i]
            t0 = cursors[i]
            if i == 0 and t0 == 48:
                order.append((i, t0, 2, "T0TAIL", False))
                load["DVE"] += 1316
                cursors[i] = tcnt
            else:
                lim = 48 if i == 0 else tcnt
                nt = min(2, lim - t0)
                if t0 == 0:
                    # tile init: direct copy into the acc (no fold) on the
                    # less-loaded PSUM-capable engine
                    if load["ACT"] + 1038 < load["DVE"] + 1192:
                        order.append((i, t0, nt, "Z0", True))
                        load["ACT"] += 1038
                    else:
                        order.append((i, t0, nt, "X", True))
                        load["DVE"] += 1192
                    cursors[i] = nt
                    continue
                best, bestkey = None, None
                for typ in ("X", "Z"):
                    m = max(max(load.values()),
                            *[load[e] + c for e, c in cost[typ][nt]])
                    key = (m, load["DVE"] + dict(cost[typ][nt]).get("DVE", 0.0))
                    if bestkey is None or key < bestkey:
                        best, bestkey = typ, key
                order.append((i, t0, nt, best, t0 == 0))
                for e, c in cost[best][nt]:
                    load[e] += c
                cursors[i] = t0 + nt
            if cursors[i] >= TILE_TCNT[i]:
                load["DVE"] += 520  # m1 max + tensor_scalar relu+bias
    return order, load


CONV_SCHED, CONV_LOAD = build_conv_schedule()

def conv_completion_order(sched):
    seen = []
    for (i, t0, nt, typ, first) in sched:
        done = (t0 + nt >= TILE_TCNT[i]) or typ == "T0TAIL"
        if done:
            seen.append(i)
    return seen

C_ORDER = conv_completion_order(CONV_SCHED)
_POS = {c: k for k, c in enumerate(C_ORDER)}
C2_ORDER = sorted(range(8), key=lambda c2: max(_POS[2 * c2], _POS[2 * c2 + 1]))
# last-completing tiles: their h8 cast gates the highway start
LATE_TILES = set(C_ORDER[-3:])



# ---------------------------------------------------------------- device IR
def build_module():
    nc = bacc.Bacc()
    SIdx = NI // 16  # 1792 int16 columns

    table = nc.dram_tensor("table", [PAD_V, 128], BF16, kind="ExternalInput")
    idx = nc.dram_tensor("idx", [128, SIdx], mybir.dt.int16, kind="ExternalInput")
    wconv = nc.dram_tensor("wconv", [KDIM, N_FILTERS], BF16, kind="ExternalInput")
    bconv = nc.dram_tensor("bconv", [128, 16], FP32, kind="ExternalInput")
    whw_nl = nc.dram_tensor("whw_nl", [2, 16, 128, 10, 128], BF16, kind="ExternalInput")
    whw_n8 = nc.dram_tensor("whw_n8", [2, 16, 128, 3, 2, 128], FP8, kind="ExternalInput")
    whw_g8 = nc.dram_tensor("whw_g8", [2, 16, 128, 8, 2, 128], FP8, kind="ExternalInput")
    bhw = nc.dram_tensor("bhw", [128, 2, 16, 2], FP32, kind="ExternalInput")
    wproj = nc.dram_tensor("wproj", [4, 128, 16, 128], BF16, kind="ExternalInput")
    bproj = nc.dram_tensor("bproj", [128, 4], FP32, kind="ExternalInput")
    out = nc.dram_tensor("out", [128, 4, TOK], FP32, kind="ExternalOutput")

    with tile.TileContext(nc) as tc:
        with (
            tc.tile_pool(name="consts", bufs=1) as cpool,
            tc.tile_pool(name="gather", bufs=2) as gpool,
            tc.tile_pool(name="xs", bufs=1) as xspool,
            tc.tile_pool(name="accs", bufs=1) as apool,
            tc.tile_pool(name="stage", bufs=4) as spool,
            tc.tile_pool(name="hbuf", bufs=2) as hpool,
            tc.tile_pool(name="wstream", bufs=2) as wpool,
            tc.tile_pool(name="small", bufs=2) as mpool,
            tc.tile_pool(name="convp", bufs=4, space="PSUM") as convp,
        ):
            # ---- constants in
            idx_t = cpool.tile([128, SIdx], mybir.dt.int16)
            nc.sync.dma_start(out=idx_t[:], in_=idx[:])
            wconv_t = cpool.tile([KDIM, N_FILTERS], BF16)
            nc.sync.dma_start(out=wconv_t[:], in_=wconv[:])
            bconv_t = cpool.tile([128, 16], FP32)
            nc.sync.dma_start(out=bconv_t[:], in_=bconv[:])
            bhw_t = cpool.tile([128, 2, 16, 2], FP32)
            nc.sync.dma_start(out=bhw_t[:], in_=bhw[:])
            bproj_t = cpool.tile([128, 4], FP32)
            nc.sync.dma_start(out=bproj_t[:], in_=bproj[:])

            # ---- 1. chunked gather + incremental im2col build
            xs = xspool.tile([KDIM, FREE], BF16)
            for (p_lo, p_hi) in CHUNK_BOUNDS:
                n_idx = (p_hi - p_lo) * TOK
                xgc = gpool.tile([128, 1, 4096], BF16, tag="xg")
                nc.gpsimd.dma_gather(
                    out_ap=xgc[:, :, 0:n_idx],
                    in_ap=table[:],
                    idxs_ap=idx_t[:, p_lo * 32:p_hi * 32],
                    num_idxs=n_idx,
                    num_idxs_reg=n_idx,
                    elem_size=128,
                    transpose=True,
                    single_packet=False,
                )
                piece_eng = ([nc.sync, nc.scalar] if p_lo == 0
                             else [nc.sync])
                for k in range(7):
                    t_lo = max(0, p_lo - k)
                    t_hi = min(NPOS, p_hi - k)
                    if t_lo >= t_hi:
                        continue
                    s0 = t_lo + k - p_lo
                    piece_eng[k % len(piece_eng)].dma_start(
                        out=xs[16 * k:16 * (k + 1), TOK * t_lo:TOK * t_hi],
                        in_=xgc[0:16, 0, TOK * s0:TOK * (s0 + (t_hi - t_lo))],
                    )

            # ---- 2+3. conv rounds (static schedule) + finales
            h1 = hpool.tile([128, 16, TOK], BF16, tag="h")
            h8a = hpool.tile([128, 16, TOK], FP8, tag="h8")
            accd = {}
            accp = {}
            accp_used = set()
            n_done = 0
            h8_on_dve = {}
            for (i, t0, nt, typ, first) in CONV_SCHED:
                lhsT = wconv_t[:, 128 * i:128 * (i + 1)]
                P = convp.tile([128, 2, TOK], FP32, tag="ps")
                for rpos in range(nt):
                    t = t0 + rpos
                    nc.tensor.matmul(
                        out=P[:, rpos, :], lhsT=lhsT,
                        rhs=xs[:, TOK * t:TOK * (t + 1)],
                        start=True, stop=True,
                    )
                if first:
                    accd[i] = apool.tile([128, 2, TOK], BF16, tag=f"accd{i}", name=f"accd{i}")
                ad = accd[i]
                if typ == "Z0":
                    nc.scalar.copy(out=ad[:, 0:nt, :], in_=P[:, 0:nt, :])
                elif typ == "T0TAIL":
                    # pos t0 valid rows 0:64 (w1+w2), pos t0+1 rows 0:32 (w1)
                    nc.vector.tensor_tensor(
                        out=ad[0:64, 0:1, :], in0=ad[0:64, 0:1, :],
                        in1=P[0:64, 0:1, :], op=mybir.AluOpType.max)
                    nc.vector.tensor_tensor(
                        out=ad[0:32, 1:2, :], in0=ad[0:32, 1:2, :],
                        in1=P[0:32, 1:2, :], op=mybir.AluOpType.max)
                elif typ == "X":
                    if first:
                        nc.vector.tensor_scalar_add(
                            out=ad[:, 0:nt, :], in0=P[:, 0:nt, :], scalar1=0.0)
                    else:
                        nc.vector.tensor_tensor(
                            out=ad[:, 0:nt, :], in0=ad[:, 0:nt, :],
                            in1=P[:, 0:nt, :], op=mybir.AluOpType.max)
                else:
                    s4 = spool.tile([128, 2, TOK], BF16, tag="s4")
                    nc.scalar.copy(out=s4[:, 0:nt, :], in_=P[:, 0:nt, :])
                    if typ == "Z":
                        if first:
                            nc.vector.tensor_scalar_add(
                                out=ad[:, 0:nt, :], in0=s4[:, 0:nt, :], scalar1=0.0)
                        else:
                            nc.vector.tensor_tensor(
                                out=ad[:, 0:nt, :], in0=ad[:, 0:nt, :],
                                in1=s4[:, 0:nt, :], op=mybir.AluOpType.max)
                    else:  # W: pool folds
                        if i not in accp:
                            accp[i] = apool.tile([128, 2, TOK], BF16, tag=f"accp{i}", name=f"accp{i}")
                            nc.gpsimd.tensor_scalar_add(
                                out=accp[i][:, 0:nt, :], in0=s4[:, 0:nt, :], scalar1=0.0)
                        else:
                            ap_ = accp[i]
                            nc.gpsimd.scalar_tensor_tensor(
                                out=ap_[:, 0:nt, :], in0=s4[:, 0:nt, :], scalar=0.0,
                                in1=ap_[:, 0:nt, :],
                                op0=mybir.AluOpType.add, op1=mybir.AluOpType.max)
                        accp_used.add(i)
                # finale when tile complete
                done = (t0 + nt >= TILE_TCNT[i]) or (typ == "T0TAIL")
                if done:
                    if i in accp:
                        nc.vector.tensor_tensor(
                            out=ad[:], in0=ad[:], in1=accp[i][:],
                            op=mybir.AluOpType.max)
                    m1 = mpool.tile([128, TOK], BF16, tag="m1")
                    nc.vector.tensor_tensor(
                        out=m1[:], in0=ad[:, 0, :], in1=ad[:, 1, :],
                        op=mybir.AluOpType.max)
                    nc.vector.tensor_scalar(
                        out=h1[:, i, :], in0=m1[:],
                        scalar1=bconv_t[:, i:i + 1], scalar2=0.0,
                        op0=mybir.AluOpType.add, op1=mybir.AluOpType.max)
                    nc.gpsimd.dma_start(out=h8a[:, i, :], in_=h1[:, i, :])
                    n_done += 1

            # ---- 4. highway layers
            h_in, h8_in = h1, h8a
            for layer in range(2):
                h_out = hpool.tile([128, 16, TOK], BF16, tag="h", name=f"h_l{layer}")
                h8_out = None
                if layer == 0:
                    h8_out = hpool.tile([128, 16, TOK], FP8, tag="h8", name="h8b")
                for j in range(16):
                    wn = wpool.tile([128, 10, 128], BF16, tag="wnh", bufs=3)
                    nc.sync.dma_start(out=wn[:], in_=whw_nl[layer, j])
                    wn8 = wpool.tile([128, 3, 2, 128], FP8, tag="wn8", bufs=3)
                    nc.sync.dma_start(out=wn8[:], in_=whw_n8[layer, j])
                    wg = wpool.tile([128, 8, 2, 128], FP8, tag="wg")
                    nc.sync.dma_start(out=wg[:], in_=whw_g8[layer, j])
                    p_nl = convp.tile([128, 2, TOK], FP32, tag="ps", name="psnl")[:, 0, :]
                    corder = [c for c in (C_ORDER if layer == 0 else range(16)) if c < 10]
                    for ci, c in enumerate(corder):
                        nc.tensor.matmul(
                            out=p_nl[:], lhsT=wn[:, c, :], rhs=h_in[:, c, :],
                            start=(ci == 0), stop=False)
                    for c2 in (5, 6, 7):
                        nc.tensor.matmul(
                            out=p_nl[:], lhsT=wn8[:, c2 - 5],
                            rhs=h8_in[:, 2 * c2:2 * c2 + 2, :],
                            start=False, stop=(c2 == 7),
                            perf_mode=mybir.MatmulPerfMode.DoubleRow)
                    p_g = convp.tile([128, 2, TOK], FP32, tag="ps", name="psg")[:, 0, :]
                    g_order = C2_ORDER if layer == 0 else list(range(8))
                    for gi, c2 in enumerate(g_order):
                        nc.tensor.matmul(
                            out=p_g[:], lhsT=wg[:, c2], rhs=h8_in[:, 2 * c2:2 * c2 + 2, :],
                            start=(gi == 0), stop=(gi == 7),
                            perf_mode=mybir.MatmulPerfMode.DoubleRow)
                    nl = mpool.tile([128, TOK], BF16, tag="nl")
                    gt = mpool.tile([128, TOK], BF16, tag="gt")
                    nc.scalar.activation(
                        out=nl[:], in_=p_nl[:],
                        func=mybir.ActivationFunctionType.Relu,
                        bias=bhw_t[:, layer, j, 0:1], scale=1.0)
                    nc.scalar.activation(
                        out=gt[:], in_=p_g[:],
                        func=mybir.ActivationFunctionType.Sigmoid,
                        bias=bhw_t[:, layer, j, 1:2], scale=1.0)
                    d = mpool.tile([128, TOK], BF16, tag="d")
                    nc.vector.tensor_tensor(
                        out=d[:], in0=h_in[:, j, :], in1=nl[:],
                        op=mybir.AluOpType.subtract)
                    nc.vector.tensor_mul(out=gt[:], in0=gt[:], in1=d[:])
                    nc.vector.tensor_add(out=h_out[:, j, :], in0=nl[:], in1=gt[:])
                    if h8_out is not None:
                        nc.gpsimd.dma_start(out=h8_out[:, j, :], in_=h_out[:, j, :])
                h_in = h_out
                if h8_out is not None:
                    h8_in = h8_out

            # ---- 5. projection, feature-major out
            for j2 in range(4):
                wp = wpool.tile([128, 16, 128], BF16, tag="wn", bufs=2)
                nc.sync.dma_start(out=wp[:], in_=wproj[j2])
                p_o = convp.tile([128, 2, TOK], FP32, tag="ps", name="pso")[:, 0, :]
                for c in range(16):
                    nc.tensor.matmul(
                        out=p_o[:], lhsT=wp[:, c, :], rhs=h_in[:, c, :],
                        start=(c == 0), stop=(c == 15))
                ot = cpool.tile([128, TOK], FP32, tag="ot", name="ot", bufs=2)
                nc.scalar.activation(
                    out=ot[:], in_=p_o[:],
                    func=mybir.ActivationFunctionType.Identity,
                    bias=bproj_t[:, j2:j2 + 1], scale=1.0)
                nc.sync.dma_start(out=out[:, j2, :], in_=ot[:])

    nc.compile()
    return nc


_CACHED = {}


def _prep(inputs):
    """Host-side layout prep: sharding, index packing, weight packing."""
    chars = np.asarray(inputs["chars"]).astype(np.int64).reshape(NTOK, L)
    chars_pad = np.full((NTOK, TP), ZERO_ROW, np.int64)
    chars_pad[:, :L] = chars

    emb = np.asarray(inputs["char_emb"], np.float32)
    table = np.zeros((PAD_V, 128), np.float32)
    table[:CHAR_VOCAB, :CHAR_DIM] = emb
    table = table.astype(ml_dtypes.bfloat16)

    wc = np.zeros((7, CHAR_DIM, N_FILTERS), np.float32)
    off = 0
    for fi, (w, n) in enumerate(FILTERS):
        cw = np.asarray(inputs[f"conv_w_{fi}"], np.float32)  # (n, 16, w)
        wc[:w, :, off:off + n] = cw.transpose(2, 1, 0)
        off += n
    wconv = wc.reshape(KDIM, N_FILTERS).astype(ml_dtypes.bfloat16)
    bconv = np.concatenate([np.asarray(inputs[f"conv_b_{i}"], np.float32)
                            for i in range(7)])
    bconv_dev = bconv.reshape(16, 128).T.copy()  # (128, 16)

    whw_nl = np.zeros((2, 16, 128, 10, 128), np.float32)
    whw_n8 = np.zeros((2, 16, 128, 3, 2, 128), np.float32)
    whw_g8 = np.zeros((2, 16, 128, 8, 2, 128), np.float32)
    bhw = np.zeros((128, 2, 16, 2), np.float32)
    for l in range(2):
        W = np.asarray(inputs[f"hw_w_{l}"], np.float32)   # (4096, 2048)
        bb = np.asarray(inputs[f"hw_b_{l}"], np.float32)  # (4096,)
        WT = W.T  # (ic 2048, oc 4096)
        # nl: (j, p, c, o) = WT[128c+p, 128j+o]; chunks 12-15 go fp8 DR
        nlv = WT[:, 0:2048].reshape(16, 128, 16, 128)       # (c, p, j, o)
        whw_nl[l] = nlv[0:10].transpose(2, 1, 0, 3)         # (j, p, c, o)
        n8v = nlv[10:16].reshape(3, 2, 128, 16, 128)        # (c2, i, p, j, o)
        whw_n8[l] = n8v.transpose(3, 2, 0, 1, 4)            # (j, p, c2, i, o)
        gv = WT[:, 2048:4096].reshape(8, 2, 128, 16, 128)   # (c2, i, p, j, o)
        whw_g8[l] = gv.transpose(3, 2, 0, 1, 4)             # (j, p, c2, i, o)
        bhw[:, l, :, 0] = bb[0:2048].reshape(16, 128).T
        bhw[:, l, :, 1] = bb[2048:4096].reshape(16, 128).T
    whw_nl = whw_nl.astype(ml_dtypes.bfloat16)
    whw_n8 = whw_n8.astype(ml_dtypes.float8_e4m3)
    whw_g8 = whw_g8.astype(ml_dtypes.float8_e4m3)

    Wp = np.asarray(inputs["proj_w"], np.float32)  # (512, 2048)
    WpT = Wp.T  # (2048, 512)
    wproj = WpT.reshape(16, 128, 4, 128).transpose(2, 1, 0, 3).copy()
    wproj = wproj.astype(ml_dtypes.bfloat16)      # (j2, p, c, o)
    bproj = np.asarray(inputs["proj_b"], np.float32).reshape(4, 128).T.copy()

    shared = dict(table=table, wconv=wconv, bconv=bconv_dev, whw_nl=whw_nl, whw_n8=whw_n8,
                  whw_g8=whw_g8, bhw=bhw, wproj=wproj, bproj=bproj)

    in_maps = []
    for core in range(N_CORES):
        cp = chars_pad[core * TOK:(core + 1) * TOK]  # (512, 56)
        idx_flat = cp.T.reshape(-1).astype(np.int16)  # j = t'*512 + n
        idx16 = idx_flat.reshape(NI // 16, 16).T.copy()  # (16, NI/16)
        idx16 = np.tile(idx16, (8, 1))  # (128, NI/16)
        m = dict(shared)
        m["idx"] = idx16
        in_maps.append(m)
    return in_maps


def kernel(**inputs) -> np.ndarray:
    if "nc" not in _CACHED:
        _CACHED["nc"] = build_module()
    nc = _CACHED["nc"]
    in_maps = _prep(inputs)
    res = run_bass_kernel_spmd(nc, in_maps, core_ids=list(range(N_CORES)))
    # out[p, j2, n] -> full[n, 128*j2 + p]
    parts = []
    for r in res.results:
        o = r["out"]  # (128, 4, 512)
        parts.append(o.transpose(2, 1, 0).reshape(TOK, PROJ_DIM))
    full = np.concatenate(parts, axis=0)
    return full.reshape(B, S, PROJ_DIM)


if __name__ == "__main__":
    order, load = CONV_SCHED, CONV_LOAD
    from collections import Counter
    print("rounds:", len(order), Counter(t for (_, _, _, t, _) in order))
    print("loads (us):", {k: round(v / 1000, 1) for k, v in load.items()})


# revision 15
# speedup vs baseline: 1.0115x; 1.0031x over previous
"""CharCNN token embedder (ELMo-style) on 8 Trainium2 NeuronCores — v2.

Data-parallel over 4096 = 16*256 tokens (512 per core), weights replicated.

Per-core pipeline (all phases overlap via tile-framework semaphores):
  1. Chunked dma_gather (8 x 3584 idx) pulls char-embedding rows into
     feature-major chunks xg_r[d, (t', n)]; per-chunk shifted copies build
     the im2col patch matrix xs[(k,d), (t, n)] incrementally, so conv
     matmuls start while later chunks are still in flight.
  2. Conv = matmuls with K=112 per 128-channel tile, one per output
     position (rounds of <=4 positions into a [128,4,512] PSUM group,
     double-buffered).  Position max-pool runs as a statically scheduled
     mix of drain types balanced across engines:
       X: DVE folds PSUM directly into a bf16 acc pair
       Z: ACT copies PSUM->bf16, DVE folds
       W: ACT copies PSUM->bf16, GPSIMD(Pool) folds (Pool cannot read PSUM)
  3. Per-tile finale: combine accs, relu+bias -> h1 (bf16) + h8 (fp8e4).
  4. 2 highway layers: nonlinear half in bf16 (16 K-chunks), gate half in
     fp8e4 DoubleRow (8 chunk-pairs, 2x PE throughput; sigmoid gating makes
     gate-half quantization error negligible).  ACT does relu/sigmoid,
     DVE does the gating arithmetic.
  5. Projection to 512 in bf16; output stored feature-major [128, 4, 512]
     fp32 and transposed on the host.
"""

import numpy as np
import ml_dtypes

import concourse.bass as bass
import concourse.mybir as mybir
import concourse.tile as tile
from concourse import bacc
from concourse.bass_utils import run_bass_kernel_spmd
from concourse.vector_clock import ScopedClock

# ---------------------------------------------------------------- constants
B, S, L = 16, 256, 50
CHAR_DIM = 16
CHAR_VOCAB = 262
PAD_V = 264
ZERO_ROW = 262
FILTERS = [(1, 32), (2, 32), (3, 64), (4, 128), (5, 256), (6, 512), (7, 1024)]
N_FILTERS = 2048
PROJ_DIM = 512
N_CORES = 8
NTOK = B * S
TOK = NTOK // N_CORES        # 512 tokens per core
TP = 56                      # padded positions per token (50 + 6)
NI = TOK * TP                # gather indices per core = 28672
# gather chunk boundaries in t'-positions: small head chunk so conv starts
# early, small tail so xg tiles stay <= 3584 indices
CHUNK_BOUNDS = [(0, 8), (8, 15), (15, 22), (22, 29),
                (29, 36), (36, 43), (43, 50), (50, 56)]
NPOS = 50
FREE = TOK * NPOS            # 25600
KDIM = 112

# per-tile valid position count (tile 0 = w1/w2/w3 mixed; see tail handling)
TILE_TCNT = [50, 47, 46, 46, 45, 45, 45, 45, 44, 44, 44, 44, 44, 44, 44, 44]

BF16 = mybir.dt.bfloat16
FP32 = mybir.dt.float32
FP8 = mybir.dt.float8e4

_MAX_WAITS_PER_INST = 1


def _patched_drain_and_barrier(self, tick_clock, wait_clock):
    # The walrus build in this container rejects CTRL instructions carrying
    # more than one sem wait; spread the kernel-tail drain waits over NOPs.
    nc = self.nc
    carrier = nc.sync.nop()
    wait_clock.add_sem_waits(carrier.ins, ScopedClock({None: tick_clock.global_clock}))
    si = carrier.ins.sync_info
    waits = list(si.on_wait) if si is not None and si.on_wait else []
    if len(waits) > _MAX_WAITS_PER_INST:
        carrier.ins.sync_info = mybir.SyncInfo(
            on_wait=waits[:_MAX_WAITS_PER_INST],
            on_update=list(si.on_update) if si.on_update else [])
        for i in range(_MAX_WAITS_PER_INST, len(waits), _MAX_WAITS_PER_INST):
            extra = nc.sync.nop()
            extra.ins.sync_info = mybir.SyncInfo(
                on_wait=waits[i:i + _MAX_WAITS_PER_INST], on_update=[])
    nc.sync.drain()
    nc.all_engine_barrier()
    assert self.sems is not None
    popped = nc._tile_sem_poison_stack.pop()
    assert popped is self._sem_poison
    nc.clear_and_free_semaphores(list(self.sems.allocated().values()))
    nc.all_engine_barrier()


tile.TileContext._drain_and_barrier = _patched_drain_and_barrier


# ------------------------------------------------------- static drain plan
def build_conv_schedule():
    """Greedy X/Z assignment of conv pooling rounds (Pool engine cannot run
    elementwise ops on TRN2, so only DVE folds PSUM directly (X) or folds
    ACT-staged bf16 copies (Z)).  Finale relu+bias runs on DVE via 4x-mode
    tensor_scalar; h8 casts ride on gpsimd cast-DMAs."""
    cost = {
        "X": {2: [("DVE", 1192)], 1: [("DVE", 658)]},
        "Z": {2: [("ACT", 1038), ("DVE", 594)], 1: [("ACT", 612), ("DVE", 387)]},
    }
    load = {"DVE": 9000.0, "ACT": 9000.0}
    fold_eng = {"X": "DVE", "Z": "DVE"}
    cursors = [0] * 16
    order = []
    while True:
        alive = [i for i in range(16) if cursors[i] < TILE_TCNT[i]]
        if not alive:
            break
        alive.sort(key=lambda i: -(TILE_TCNT[i] - cursors[i]))
        for i in alive:
            tcnt = TILE_TCNT[i]
            t0 = cursors[i]
            if i == 0 and t0 == 48:
                order.append((i, t0, 2, "T0TAIL", False))
                load["DVE"] += 1316
                cursors[i] = tcnt
            else:
                lim = 48 if i == 0 else tcnt
                nt = min(2, lim - t0)
                if t0 == 0:
                    # tile init: direct copy into the acc (no fold) on the
                    # less-loaded PSUM-capable engine
                    if load["ACT"] + 1038 < load["DVE"] + 1192:
                        order.append((i, t0, nt, "Z0", True))
                        load["ACT"] += 1038
                    else:
                        order.append((i, t0, nt, "X", True))
                        load["DVE"] += 1192
                    cursors[i] = nt
                    continue
                best, bestkey = None, None
                for typ in ("X", "Z"):
                    m = max(max(load.values()),
                            *[load[e] + c for e, c in cost[typ][nt]])
                    key = (m, load["DVE"] + dict(cost[typ][nt]).get("DVE", 0.0))
                    if bestkey is None or key < bestkey:
                        best, bestkey = typ, key
                order.append((i, t0, nt, best, t0 == 0))
                for e, c in cost[best][nt]:
                    load[e] += c
                cursors[i] = t0 + nt
            if cursors[i] >= TILE_TCNT[i]:
                load["DVE"] += 520  # m1 max + tensor_scalar relu+bias
    return order, load


CONV_SCHED, CONV_LOAD = build_conv_schedule()

def conv_completion_order(sched):
    seen = []
    for (i, t0, nt, typ, first) in sched:
        done = (t0 + nt >= TILE_TCNT[i]) or typ == "T0TAIL"
        if done:
            seen.append(i)
    return seen

C_ORDER = conv_completion_order(CONV_SCHED)
_POS = {c: k for k, c in enumerate(C_ORDER)}
C2_ORDER = sorted(range(8), key=lambda c2: max(_POS[2 * c2], _POS[2 * c2 + 1]))
# last-completing tiles: their h8 cast gates the highway start
LATE_TILES = set(C_ORDER[-3:])



# ---------------------------------------------------------------- device IR
def build_module():
    nc = bacc.Bacc()
    SIdx = NI // 16  # 1792 int16 columns

    table = nc.dram_tensor("table", [PAD_V, 128], BF16, kind="ExternalInput")
    idx = nc.dram_tensor("idx", [128, SIdx], mybir.dt.int16, kind="ExternalInput")
    wconv = nc.dram_tensor("wconv", [KDIM, N_FILTERS], BF16, kind="ExternalInput")
    bconv = nc.dram_tensor("bconv", [128, 16], FP32, kind="ExternalInput")
    whw_nl = nc.dram_tensor("whw_nl", [2, 16, 128, 10, 128], BF16, kind="ExternalInput")
    whw_n8 = nc.dram_tensor("whw_n8", [2, 16, 128, 3, 2, 128], FP8, kind="ExternalInput")
    whw_g8 = nc.dram_tensor("whw_g8", [2, 16, 128, 8, 2, 128], FP8, kind="ExternalInput")
    bhw = nc.dram_tensor("bhw", [128, 2, 16, 2], FP32, kind="ExternalInput")
    wproj = nc.dram_tensor("wproj", [4, 128, 16, 128], BF16, kind="ExternalInput")
    bproj = nc.dram_tensor("bproj", [128, 4], FP32, kind="ExternalInput")
    out = nc.dram_tensor("out", [128, 4, TOK], FP32, kind="ExternalOutput")

    with tile.TileContext(nc) as tc:
        with (
            tc.tile_pool(name="consts", bufs=1) as cpool,
            tc.tile_pool(name="gather", bufs=2) as gpool,
            tc.tile_pool(name="xs", bufs=1) as xspool,
            tc.tile_pool(name="accs", bufs=1) as apool,
            tc.tile_pool(name="stage", bufs=4) as spool,
            tc.tile_pool(name="hbuf", bufs=2) as hpool,
            tc.tile_pool(name="wstream", bufs=2) as wpool,
            tc.tile_pool(name="small", bufs=2) as mpool,
            tc.tile_pool(name="convp", bufs=4, space="PSUM") as convp,
        ):
            # ---- constants in
            idx_t = cpool.tile([128, SIdx], mybir.dt.int16)
            nc.sync.dma_start(out=idx_t[:], in_=idx[:])
            wconv_t = cpool.tile([KDIM, N_FILTERS], BF16)
            nc.sync.dma_start(out=wconv_t[:], in_=wconv[:])
            bconv_t = cpool.tile([128, 16], FP32)
            nc.sync.dma_start(out=bconv_t[:], in_=bconv[:])
            bhw_t = cpool.tile([128, 2, 16, 2], FP32)
            nc.sync.dma_start(out=bhw_t[:], in_=bhw[:])
            bproj_t = cpool.tile([128, 4], FP32)
            nc.sync.dma_start(out=bproj_t[:], in_=bproj[:])

            # ---- 1. chunked gather + incremental im2col build
            xs = xspool.tile([KDIM, FREE], BF16)
            for (p_lo, p_hi) in CHUNK_BOUNDS:
                n_idx = (p_hi - p_lo) * TOK
                xgc = gpool.tile([128, 1, 4096], BF16, tag="xg")
                nc.gpsimd.dma_gather(
                    out_ap=xgc[:, :, 0:n_idx],
                    in_ap=table[:],
                    idxs_ap=idx_t[:, p_lo * 32:p_hi * 32],
                    num_idxs=n_idx,
                    num_idxs_reg=n_idx,
                    elem_size=128,
                    transpose=True,
                    single_packet=False,
                )
                piece_eng = ([nc.sync, nc.scalar] if p_lo == 0
                             else [nc.sync])
                for k in range(7):
                    t_lo = max(0, p_lo - k)
                    t_hi = min(NPOS, p_hi - k)
                    if t_lo >= t_hi:
                        continue
                    s0 = t_lo + k - p_lo
                    piece_eng[k % len(piece_eng)].dma_start(
                        out=xs[16 * k:16 * (k + 1), TOK * t_lo:TOK * t_hi],
                        in_=xgc[0:16, 0, TOK * s0:TOK * (s0 + (t_hi - t_lo))],
                    )

            # ---- 2+3. conv rounds (static schedule) + finales
            h1 = hpool.tile([128, 16, TOK], BF16, tag="h")
            h8a = hpool.tile([128, 16, TOK], FP8, tag="h8")
            accd = {}
            accp = {}
            accp_used = set()
            n_done = 0
            h8_on_dve = {}
            for (i, t0, nt, typ, first) in CONV_SCHED:
                lhsT = wconv_t[:, 128 * i:128 * (i + 1)]
                P = convp.tile([128, 2, TOK], FP32, tag="ps")
                for rpos in range(nt):
                    t = t0 + rpos
                    nc.tensor.matmul(
                        out=P[:, rpos, :], lhsT=lhsT,
                        rhs=xs[:, TOK * t:TOK * (t + 1)],
                        start=True, stop=True,
                    )
                if first:
                    accd[i] = apool.tile([128, 2, TOK], BF16, tag=f"accd{i}", name=f"accd{i}")
                ad = accd[i]
                if typ == "Z0":
                    nc.scalar.copy(out=ad[:, 0:nt, :], in_=P[:, 0:nt, :])
                elif typ == "T0TAIL":
                    # pos t0 valid rows 0:64 (w1+w2), pos t0+1 rows 0:32 (w1)
                    nc.vector.tensor_tensor(
                        out=ad[0:64, 0:1, :], in0=ad[0:64, 0:1, :],
                        in1=P[0:64, 0:1, :], op=mybir.AluOpType.max)
                    nc.vector.tensor_tensor(
                        out=ad[0:32, 1:2, :], in0=ad[0:32, 1:2, :],
                        in1=P[0:32, 1:2, :], op=mybir.AluOpType.max)
                elif typ == "X":
                    if first:
                        nc.vector.tensor_scalar_add(
                            out=ad[:, 0:nt, :], in0=P[:, 0:nt, :], scalar1=0.0)
                    else:
                        nc.vector.tensor_tensor(
                            out=ad[:, 0:nt, :], in0=ad[:, 0:nt, :],
                            in1=P[:, 0:nt, :], op=mybir.AluOpType.max)
                else:
                    s4 = spool.tile([128, 2, TOK], BF16, tag="s4")
                    nc.scalar.copy(out=s4[:, 0:nt, :], in_=P[:, 0:nt, :])
                    if typ == "Z":
                        if first:
                            nc.vector.tensor_scalar_add(
                                out=ad[:, 0:nt, :], in0=s4[:, 0:nt, :], scalar1=0.0)
                        else:
                            nc.vector.tensor_tensor(
                                out=ad[:, 0:nt, :], in0=ad[:, 0:nt, :],
                                in1=s4[:, 0:nt, :], op=mybir.AluOpType.max)
                    else:  # W: pool folds
                        if i not in accp:
                            accp[i] = apool.tile([128, 2, TOK], BF16, tag=f"accp{i}", name=f"accp{i}")
                            nc.gpsimd.tensor_scalar_add(
                                out=accp[i][:, 0:nt, :], in0=s4[:, 0:nt, :], scalar1=0.0)
                        else:
                            ap_ = accp[i]
                            nc.gpsimd.scalar_tensor_tensor(
                                out=ap_[:, 0:nt, :], in0=s4[:, 0:nt, :], scalar=0.0,
                                in1=ap_[:, 0:nt, :],
                                op0=mybir.AluOpType.add, op1=mybir.AluOpType.max)
                        accp_used.add(i)
                # finale when tile complete
                done = (t0 + nt >= TILE_TCNT[i]) or (typ == "T0TAIL")
                if done:
                    if i in accp:
                        nc.vector.tensor_tensor(
                            out=ad[:], in0=ad[:], in1=accp[i][:],
                            op=mybir.AluOpType.max)
                    m1 = mpool.tile([128, TOK], BF16, tag="m1")
                    nc.vector.tensor_tensor(
                        out=m1[:], in0=ad[:, 0, :], in1=ad[:, 1, :],
                        op=mybir.AluOpType.max)
                    nc.vector.tensor_scalar(
                        out=h1[:, i, :], in0=m1[:],
                        scalar1=bconv_t[:, i:i + 1], scalar2=0.0,
                        op0=mybir.AluOpType.add, op1=mybir.AluOpType.max)
                    nc.gpsimd.dma_start(out=h8a[:, i, :], in_=h1[:, i, :])
                    n_done += 1

            # ---- 4. highway layers
            h_in, h8_in = h1, h8a
            for layer in range(2):
                h_out = hpool.tile([128, 16, TOK], BF16, tag="h", name=f"h_l{layer}")
                h8_out = None
                if layer == 0:
                    h8_out = hpool.tile([128, 16, TOK], FP8, tag="h8", name="h8b")
                for j in range(16):
                    wn = wpool.tile([128, 10, 128], BF16, tag="wnh", bufs=3)
                    nc.sync.dma_start(out=wn[:], in_=whw_nl[layer, j])
                    wn8 = wpool.tile([128, 3, 2, 128], FP8, tag="wn8", bufs=3)
                    nc.sync.dma_start(out=wn8[:], in_=whw_n8[layer, j])
                    wg = wpool.tile([128, 8, 2, 128], FP8, tag="wg")
                    nc.sync.dma_start(out=wg[:], in_=whw_g8[layer, j])
                    p_nl = convp.tile([128, 2, TOK], FP32, tag="ps", name="psnl")[:, 0, :]
                    corder = [c for c in (C_ORDER if layer == 0 else range(16)) if c < 10]
                    for ci, c in enumerate(corder):
                        nc.tensor.matmul(
                            out=p_nl[:], lhsT=wn[:, c, :], rhs=h_in[:, c, :],
                            start=(ci == 0), stop=False)
                    for c2 in (5, 6, 7):
                        nc.tensor.matmul(
                            out=p_nl[:], lhsT=wn8[:, c2 - 5],
                            rhs=h8_in[:, 2 * c2:2 * c2 + 2, :],
                            start=False, stop=(c2 == 7),
                            perf_mode=mybir.MatmulPerfMode.DoubleRow)
                    p_g = convp.tile([128, 2, TOK], FP32, tag="ps", name="psg")[:, 0, :]
                    g_order = C2_ORDER if layer == 0 else list(range(8))
                    for gi, c2 in enumerate(g_order):
                        nc.tensor.matmul(
                            out=p_g[:], lhsT=wg[:, c2], rhs=h8_in[:, 2 * c2:2 * c2 + 2, :],
                            start=(gi == 0), stop=(gi == 7),
                            perf_mode=mybir.MatmulPerfMode.DoubleRow)
                    nl = mpool.tile([128, TOK], BF16, tag="nl")
                    gt = mpool.tile([128, TOK], BF16, tag="gt")
                    nc.scalar.activation(
                        out=nl[:], in_=p_nl[:],
                        func=mybir.ActivationFunctionType.Relu,
                        bias=bhw_t[:, layer, j, 0:1], scale=1.0)
                    nc.scalar.activation(
                        out=gt[:], in_=p_g[:],
                        func=mybir.ActivationFunctionType.Sigmoid,
                        bias=bhw_t[:, layer, j, 1:2], scale=1.0)
                    d = mpool.tile([128, TOK], BF16, tag="d")
                    nc.vector.tensor_tensor(
                        out=d[:], in0=h_in[:, j, :], in1=nl[:],
                        op=mybir.AluOpType.subtract)
                    nc.vector.tensor_mul(out=gt[:], in0=gt[:], in1=d[:])
                    nc.vector.tensor_add(out=h_out[:, j, :], in0=nl[:], in1=gt[:])
                    if h8_out is not None:
                        if j >= 13:
                            nc.vector.tensor_scalar_add(
                                out=h8_out[:, j, :], in0=h_out[:, j, :], scalar1=0.0)
                        else:
                            nc.gpsimd.dma_start(out=h8_out[:, j, :], in_=h_out[:, j, :])
                h_in = h_out
                if h8_out is not None:
                    h8_in = h8_out

            # ---- 5. projection, feature-major out
            for j2 in range(4):
                wp = wpool.tile([128, 16, 128], BF16, tag="wn", bufs=2)
                nc.sync.dma_start(out=wp[:], in_=wproj[j2])
                p_o = convp.tile([128, 2, TOK], FP32, tag="ps", name="pso")[:, 0, :]
                for c in range(16):
                    nc.tensor.matmul(
                        out=p_o[:], lhsT=wp[:, c, :], rhs=h_in[:, c, :],
                        start=(c == 0), stop=(c == 15))
                ot = cpool.tile([128, TOK], FP32, tag="ot", name="ot", bufs=2)
                nc.scalar.activation(
                    out=ot[:], in_=p_o[:],
                    func=mybir.ActivationFunctionType.Identity,
                    bias=bproj_t[:, j2:j2 + 1], scale=1.0)
                nc.sync.dma_start(out=out[:, j2, :], in_=ot[:])

    nc.compile()
    return nc


_CACHED = {}


def _prep(inputs):
    """Host-side layout prep: sharding, index packing, weight packing."""
    chars = np.asarray(inputs["chars"]).astype(np.int64).reshape(NTOK, L)
    chars_pad = np.full((NTOK, TP), ZERO_ROW, np.int64)
    chars_pad[:, :L] = chars

    emb = np.asarray(inputs["char_emb"], np.float32)
    table = np.zeros((PAD_V, 128), np.float32)
    table[:CHAR_VOCAB, :CHAR_DIM] = emb
    table = table.astype(ml_dtypes.bfloat16)

    wc = np.zeros((7, CHAR_DIM, N_FILTERS), np.float32)
    off = 0
    for fi, (w, n) in enumerate(FILTERS):
        cw = np.asarray(inputs[f"conv_w_{fi}"], np.float32)  # (n, 16, w)
        wc[:w, :, off:off + n] = cw.transpose(2, 1, 0)
        off += n
    wconv = wc.reshape(KDIM, N_FILTERS).astype(ml_dtypes.bfloat16)
    bconv = np.concatenate([np.asarray(inputs[f"conv_b_{i}"], np.float32)
                            for i in range(7)])
    bconv_dev = bconv.reshape(16, 128).T.copy()  # (128, 16)

    whw_nl = np.zeros((2, 16, 128, 10, 128), np.float32)
    whw_n8 = np.zeros((2, 16, 128, 3, 2, 128), np.float32)
    whw_g8 = np.zeros((2, 16, 128, 8, 2, 128), np.float32)
    bhw = np.zeros((128, 2, 16, 2), np.float32)
    for l in range(2):
        W = np.asarray(inputs[f"hw_w_{l}"], np.float32)   # (4096, 2048)
        bb = np.asarray(inputs[f"hw_b_{l}"], np.float32)  # (4096,)
        WT = W.T  # (ic 2048, oc 4096)
        # nl: (j, p, c, o) = WT[128c+p, 128j+o]; chunks 12-15 go fp8 DR
        nlv = WT[:, 0:2048].reshape(16, 128, 16, 128)       # (c, p, j, o)
        whw_nl[l] = nlv[0:10].transpose(2, 1, 0, 3)         # (j, p, c, o)
        n8v = nlv[10:16].reshape(3, 2, 128, 16, 128)        # (c2, i, p, j, o)
        whw_n8[l] = n8v.transpose(3, 2, 0, 1, 4)            # (j, p, c2, i, o)
        gv = WT[:, 2048:4096].reshape(8, 2, 128, 16, 128)   # (c2, i, p, j, o)
        whw_g8[l] = gv.transpose(3, 2, 0, 1, 4)             # (j, p, c2, i, o)
        bhw[:, l, :, 0] = bb[0:2048].reshape(16, 128).T
        bhw[:, l, :, 1] = bb[2048:4096].reshape(16, 128).T
    whw_nl = whw_nl.astype(ml_dtypes.bfloat16)
    whw_n8 = whw_n8.astype(ml_dtypes.float8_e4m3)
    whw_g8 = whw_g8.astype(ml_dtypes.float8_e4m3)

    Wp = np.asarray(inputs["proj_w"], np.float32)  # (512, 2048)
    WpT = Wp.T  # (2048, 512)
    wproj = WpT.reshape(16, 128, 4, 128).transpose(2, 1, 0, 3).copy()
    wproj = wproj.astype(ml_dtypes.bfloat16)      # (j2, p, c, o)
    bproj = np.asarray(inputs["proj_b"], np.float32).reshape(4, 128).T.copy()

    shared = dict(table=table, wconv=wconv, bconv=bconv_dev, whw_nl=whw_nl, whw_n8=whw_n8,
                  whw_g8=whw_g8, bhw=bhw, wproj=wproj, bproj=bproj)

    in_maps = []
    for core in range(N_CORES):
        cp = chars_pad[core * TOK:(core + 1) * TOK]  # (512, 56)
        idx_flat = cp.T.reshape(-1).astype(np.int16)  # j = t'*512 + n
        idx16 = idx_flat.reshape(NI // 16, 16).T.copy()  # (16, NI/16)
        idx16 = np.tile(idx16, (8, 1))  # (128, NI/16)
        m = dict(shared)
        m["idx"] = idx16
        in_maps.append(m)
    return in_maps


def kernel(**inputs) -> np.ndarray:
    if "nc" not in _CACHED:
        _CACHED["nc"] = build_module()
    nc = _CACHED["nc"]
    in_maps = _prep(inputs)
    res = run_bass_kernel_spmd(nc, in_maps, core_ids=list(range(N_CORES)))
    # out[p, j2, n] -> full[n, 128*j2 + p]
    parts = []
    for r in res.results:
        o = r["out"]  # (128, 4, 512)
        parts.append(o.transpose(2, 1, 0).reshape(TOK, PROJ_DIM))
    full = np.concatenate(parts, axis=0)
    return full.reshape(B, S, PROJ_DIM)


if __name__ == "__main__":
    order, load = CONV_SCHED, CONV_LOAD
    from collections import Counter
    print("rounds:", len(order), Counter(t for (_, _, _, t, _) in order))
    print("loads (us):", {k: round(v / 1000, 1) for k, v in load.items()})


# revision 16
# speedup vs baseline: 1.0127x; 1.0012x over previous
"""CharCNN token embedder (ELMo-style) on 8 Trainium2 NeuronCores — v2.

Data-parallel over 4096 = 16*256 tokens (512 per core), weights replicated.

Per-core pipeline (all phases overlap via tile-framework semaphores):
  1. Chunked dma_gather (8 x 3584 idx) pulls char-embedding rows into
     feature-major chunks xg_r[d, (t', n)]; per-chunk shifted copies build
     the im2col patch matrix xs[(k,d), (t, n)] incrementally, so conv
     matmuls start while later chunks are still in flight.
  2. Conv = matmuls with K=112 per 128-channel tile, one per output
     position (rounds of <=4 positions into a [128,4,512] PSUM group,
     double-buffered).  Position max-pool runs as a statically scheduled
     mix of drain types balanced across engines:
       X: DVE folds PSUM directly into a bf16 acc pair
       Z: ACT copies PSUM->bf16, DVE folds
       W: ACT copies PSUM->bf16, GPSIMD(Pool) folds (Pool cannot read PSUM)
  3. Per-tile finale: combine accs, relu+bias -> h1 (bf16) + h8 (fp8e4).
  4. 2 highway layers: nonlinear half in bf16 (16 K-chunks), gate half in
     fp8e4 DoubleRow (8 chunk-pairs, 2x PE throughput; sigmoid gating makes
     gate-half quantization error negligible).  ACT does relu/sigmoid,
     DVE does the gating arithmetic.
  5. Projection to 512 in bf16; output stored feature-major [128, 4, 512]
     fp32 and transposed on the host.
"""

import numpy as np
import ml_dtypes

import concourse.bass as bass
import concourse.mybir as mybir
import concourse.tile as tile
from concourse import bacc
from concourse.bass_utils import run_bass_kernel_spmd
from concourse.vector_clock import ScopedClock

# ---------------------------------------------------------------- constants
B, S, L = 16, 256, 50
CHAR_DIM = 16
CHAR_VOCAB = 262
PAD_V = 264
ZERO_ROW = 262
FILTERS = [(1, 32), (2, 32), (3, 64), (4, 128), (5, 256), (6, 512), (7, 1024)]
N_FILTERS = 2048
PROJ_DIM = 512
N_CORES = 8
NTOK = B * S
TOK = NTOK // N_CORES        # 512 tokens per core
TP = 56                      # padded positions per token (50 + 6)
NI = TOK * TP                # gather indices per core = 28672
# gather chunk boundaries in t'-positions: small head chunk so conv starts
# early, small tail so xg tiles stay <= 3584 indices
CHUNK_BOUNDS = [(0, 8), (8, 15), (15, 22), (22, 29),
                (29, 36), (36, 43), (43, 50), (50, 56)]
NPOS = 50
FREE = TOK * NPOS            # 25600
KDIM = 112

# per-tile valid position count (tile 0 = w1/w2/w3 mixed; see tail handling)
TILE_TCNT = [50, 47, 46, 46, 45, 45, 45, 45, 44, 44, 44, 44, 44, 44, 44, 44]

BF16 = mybir.dt.bfloat16
FP32 = mybir.dt.float32
FP8 = mybir.dt.float8e4

_MAX_WAITS_PER_INST = 1


def _patched_drain_and_barrier(self, tick_clock, wait_clock):
    # The walrus build in this container rejects CTRL instructions carrying
    # more than one sem wait; spread the kernel-tail drain waits over NOPs.
    nc = self.nc
    carrier = nc.sync.nop()
    wait_clock.add_sem_waits(carrier.ins, ScopedClock({None: tick_clock.global_clock}))
    si = carrier.ins.sync_info
    waits = list(si.on_wait) if si is not None and si.on_wait else []
    if len(waits) > _MAX_WAITS_PER_INST:
        carrier.ins.sync_info = mybir.SyncInfo(
            on_wait=waits[:_MAX_WAITS_PER_INST],
            on_update=list(si.on_update) if si.on_update else [])
        for i in range(_MAX_WAITS_PER_INST, len(waits), _MAX_WAITS_PER_INST):
            extra = nc.sync.nop()
            extra.ins.sync_info = mybir.SyncInfo(
                on_wait=waits[i:i + _MAX_WAITS_PER_INST], on_update=[])
    nc.sync.drain()
    nc.all_engine_barrier()
    assert self.sems is not None
    popped = nc._tile_sem_poison_stack.pop()
    assert popped is self._sem_poison
    nc.clear_and_free_semaphores(list(self.sems.allocated().values()))
    nc.all_engine_barrier()


tile.TileContext._drain_and_barrier = _patched_drain_and_barrier


# ------------------------------------------------------- static drain plan
def build_conv_schedule():
    """Greedy X/Z assignment of conv pooling rounds (Pool engine cannot run
    elementwise ops on TRN2, so only DVE folds PSUM directly (X) or folds
    ACT-staged bf16 copies (Z)).  Finale relu+bias runs on DVE via 4x-mode
    tensor_scalar; h8 casts ride on gpsimd cast-DMAs."""
    cost = {
        "X": {2: [("DVE", 1192)], 1: [("DVE", 658)]},
        "Z": {2: [("ACT", 1038), ("DVE", 594)], 1: [("ACT", 612), ("DVE", 387)]},
    }
    load = {"DVE": 9000.0, "ACT": 9000.0}
    fold_eng = {"X": "DVE", "Z": "DVE"}
    cursors = [0] * 16
    order = []
    while True:
        alive = [i for i in range(16) if cursors[i] < TILE_TCNT[i]]
        if not alive:
            break
        alive.sort(key=lambda i: -(TILE_TCNT[i] - cursors[i]))
        for i in alive:
            tcnt = TILE_TCNT[i]
            t0 = cursors[i]
            if i == 0 and t0 == 48:
                order.append((i, t0, 2, "T0TAIL", False))
                load["DVE"] += 1316
                cursors[i] = tcnt
            else:
                lim = 48 if i == 0 else tcnt
                nt = min(2, lim - t0)
                if t0 == 0:
                    # tile init: direct copy into the acc (no fold) on the
                    # less-loaded PSUM-capable engine
                    if load["ACT"] + 1038 < load["DVE"] + 1192:
                        order.append((i, t0, nt, "Z0", True))
                        load["ACT"] += 1038
                    else:
                        order.append((i, t0, nt, "X", True))
                        load["DVE"] += 1192
                    cursors[i] = nt
                    continue
                best, bestkey = None, None
                for typ in ("X", "Z"):
                    m = max(max(load.values()),
                            *[load[e] + c for e, c in cost[typ][nt]])
                    key = (m, load["DVE"] + dict(cost[typ][nt]).get("DVE", 0.0))
                    if bestkey is None or key < bestkey:
                        best, bestkey = typ, key
                order.append((i, t0, nt, best, t0 == 0))
                for e, c in cost[best][nt]:
                    load[e] += c
                cursors[i] = t0 + nt
            if cursors[i] >= TILE_TCNT[i]:
                load["DVE"] += 520  # m1 max + tensor_scalar relu+bias
    return order, load


CONV_SCHED, CONV_LOAD = build_conv_schedule()

def conv_completion_order(sched):
    seen = []
    for (i, t0, nt, typ, first) in sched:
        done = (t0 + nt >= TILE_TCNT[i]) or typ == "T0TAIL"
        if done:
            seen.append(i)
    return seen

C_ORDER = conv_completion_order(CONV_SCHED)
_POS = {c: k for k, c in enumerate(C_ORDER)}
C2_ORDER = sorted(range(8), key=lambda c2: max(_POS[2 * c2], _POS[2 * c2 + 1]))
# last-completing tiles: their h8 cast gates the highway start
LATE_TILES = set(C_ORDER[-3:])



# ---------------------------------------------------------------- device IR
def build_module():
    nc = bacc.Bacc()
    SIdx = NI // 16  # 1792 int16 columns

    table = nc.dram_tensor("table", [PAD_V, 128], BF16, kind="ExternalInput")
    idx = nc.dram_tensor("idx", [128, SIdx], mybir.dt.int16, kind="ExternalInput")
    wconv = nc.dram_tensor("wconv", [KDIM, N_FILTERS], BF16, kind="ExternalInput")
    bconv = nc.dram_tensor("bconv", [128, 16], FP32, kind="ExternalInput")
    whw_nl = nc.dram_tensor("whw_nl", [2, 16, 128, 10, 128], BF16, kind="ExternalInput")
    whw_n8 = nc.dram_tensor("whw_n8", [2, 16, 128, 3, 2, 128], FP8, kind="ExternalInput")
    whw_g8 = nc.dram_tensor("whw_g8", [2, 16, 128, 8, 2, 128], FP8, kind="ExternalInput")
    bhw = nc.dram_tensor("bhw", [128, 2, 16, 2], FP32, kind="ExternalInput")
    wproj = nc.dram_tensor("wproj", [4, 128, 16, 128], BF16, kind="ExternalInput")
    bproj = nc.dram_tensor("bproj", [128, 4], FP32, kind="ExternalInput")
    out = nc.dram_tensor("out", [128, 4, TOK], FP32, kind="ExternalOutput")

    with tile.TileContext(nc) as tc:
        with (
            tc.tile_pool(name="consts", bufs=1) as cpool,
            tc.tile_pool(name="gather", bufs=2) as gpool,
            tc.tile_pool(name="xs", bufs=1) as xspool,
            tc.tile_pool(name="accs", bufs=1) as apool,
            tc.tile_pool(name="stage", bufs=4) as spool,
            tc.tile_pool(name="hbuf", bufs=2) as hpool,
            tc.tile_pool(name="wstream", bufs=2) as wpool,
            tc.tile_pool(name="small", bufs=2) as mpool,
            tc.tile_pool(name="convp", bufs=4, space="PSUM") as convp,
        ):
            # ---- constants in
            idx_t = cpool.tile([128, SIdx], mybir.dt.int16)
            nc.sync.dma_start(out=idx_t[:, 0:256], in_=idx[:, 0:256])
            nc.sync.dma_start(out=idx_t[:, 256:], in_=idx[:, 256:])
            wconv_t = cpool.tile([KDIM, N_FILTERS], BF16)
            nc.sync.dma_start(out=wconv_t[:], in_=wconv[:])
            bconv_t = cpool.tile([128, 16], FP32)
            nc.sync.dma_start(out=bconv_t[:], in_=bconv[:])
            bhw_t = cpool.tile([128, 2, 16, 2], FP32)
            nc.sync.dma_start(out=bhw_t[:], in_=bhw[:])
            bproj_t = cpool.tile([128, 4], FP32)
            nc.sync.dma_start(out=bproj_t[:], in_=bproj[:])

            # ---- 1. chunked gather + incremental im2col build
            # PE p-state warmup: dummy matmuls right after wconv lands so the
            # ramp clock has matured before the first real conv matmul
            Pw = convp.tile([128, 2, TOK], FP32, tag="ps", name="warm")
            for wi in range(6):
                nc.tensor.matmul(
                    out=Pw[:, wi % 2, :], lhsT=wconv_t[:, 0:128],
                    rhs=wconv_t[:, 512:1024], start=True, stop=True)
            xs = xspool.tile([KDIM, FREE], BF16)
            for (p_lo, p_hi) in CHUNK_BOUNDS:
                n_idx = (p_hi - p_lo) * TOK
                xgc = gpool.tile([128, 1, 4096], BF16, tag="xg")
                nc.gpsimd.dma_gather(
                    out_ap=xgc[:, :, 0:n_idx],
                    in_ap=table[:],
                    idxs_ap=idx_t[:, p_lo * 32:p_hi * 32],
                    num_idxs=n_idx,
                    num_idxs_reg=n_idx,
                    elem_size=128,
                    transpose=True,
                    single_packet=False,
                )
                piece_eng = ([nc.sync, nc.scalar] if p_lo == 0
                             else [nc.sync])
                for k in range(7):
                    t_lo = max(0, p_lo - k)
                    t_hi = min(NPOS, p_hi - k)
                    if t_lo >= t_hi:
                        continue
                    s0 = t_lo + k - p_lo
                    piece_eng[k % len(piece_eng)].dma_start(
                        out=xs[16 * k:16 * (k + 1), TOK * t_lo:TOK * t_hi],
                        in_=xgc[0:16, 0, TOK * s0:TOK * (s0 + (t_hi - t_lo))],
                    )

            # ---- 2+3. conv rounds (static schedule) + finales
            h1 = hpool.tile([128, 16, TOK], BF16, tag="h")
            h8a = hpool.tile([128, 16, TOK], FP8, tag="h8")
            accd = {}
            accp = {}
            accp_used = set()
            n_done = 0
            h8_on_dve = {}
            for (i, t0, nt, typ, first) in CONV_SCHED:
                lhsT = wconv_t[:, 128 * i:128 * (i + 1)]
                P = convp.tile([128, 2, TOK], FP32, tag="ps")
                for rpos in range(nt):
                    t = t0 + rpos
                    nc.tensor.matmul(
                        out=P[:, rpos, :], lhsT=lhsT,
                        rhs=xs[:, TOK * t:TOK * (t + 1)],
                        start=True, stop=True,
                    )
                if first:
                    accd[i] = apool.tile([128, 2, TOK], BF16, tag=f"accd{i}", name=f"accd{i}")
                ad = accd[i]
                if typ == "Z0":
                    nc.scalar.copy(out=ad[:, 0:nt, :], in_=P[:, 0:nt, :])
                elif typ == "T0TAIL":
                    # pos t0 valid rows 0:64 (w1+w2), pos t0+1 rows 0:32 (w1)
                    nc.vector.tensor_tensor(
                        out=ad[0:64, 0:1, :], in0=ad[0:64, 0:1, :],
                        in1=P[0:64, 0:1, :], op=mybir.AluOpType.max)
                    nc.vector.tensor_tensor(
                        out=ad[0:32, 1:2, :], in0=ad[0:32, 1:2, :],
                        in1=P[0:32, 1:2, :], op=mybir.AluOpType.max)
                elif typ == "X":
                    if first:
                        nc.vector.tensor_scalar_add(
                            out=ad[:, 0:nt, :], in0=P[:, 0:nt, :], scalar1=0.0)
                    else:
                        nc.vector.tensor_tensor(
                            out=ad[:, 0:nt, :], in0=ad[:, 0:nt, :],
                            in1=P[:, 0:nt, :], op=mybir.AluOpType.max)
                else:
                    s4 = spool.tile([128, 2, TOK], BF16, tag="s4")
                    nc.scalar.copy(out=s4[:, 0:nt, :], in_=P[:, 0:nt, :])
                    if typ == "Z":
                        if first:
                            nc.vector.tensor_scalar_add(
                                out=ad[:, 0:nt, :], in0=s4[:, 0:nt, :], scalar1=0.0)
                        else:
                            nc.vector.tensor_tensor(
                                out=ad[:, 0:nt, :], in0=ad[:, 0:nt, :],
                                in1=s4[:, 0:nt, :], op=mybir.AluOpType.max)
                    else:  # W: pool folds
                        if i not in accp:
                            accp[i] = apool.tile([128, 2, TOK], BF16, tag=f"accp{i}", name=f"accp{i}")
                            nc.gpsimd.tensor_scalar_add(
                                out=accp[i][:, 0:nt, :], in0=s4[:, 0:nt, :], scalar1=0.0)
                        else:
                            ap_ = accp[i]
                            nc.gpsimd.scalar_tensor_tensor(
                                out=ap_[:, 0:nt, :], in0=s4[:, 0:nt, :], scalar=0.0,
                                in1=ap_[:, 0:nt, :],
                                op0=mybir.AluOpType.add, op1=mybir.AluOpType.max)
                        accp_used.add(i)
                # finale when tile complete
                done = (t0 + nt >= TILE_TCNT[i]) or (typ == "T0TAIL")
                if done:
                    if i in accp:
                        nc.vector.tensor_tensor(
                            out=ad[:], in0=ad[:], in1=accp[i][:],
                            op=mybir.AluOpType.max)
                    m1 = mpool.tile([128, TOK], BF16, tag="m1")
                    nc.vector.tensor_tensor(
                        out=m1[:], in0=ad[:, 0, :], in1=ad[:, 1, :],
                        op=mybir.AluOpType.max)
                    nc.vector.tensor_scalar(
                        out=h1[:, i, :], in0=m1[:],
                        scalar1=bconv_t[:, i:i + 1], scalar2=0.0,
                        op0=mybir.AluOpType.add, op1=mybir.AluOpType.max)
                    nc.gpsimd.dma_start(out=h8a[:, i, :], in_=h1[:, i, :])
                    n_done += 1

            # ---- 4. highway layers
            h_in, h8_in = h1, h8a
            for layer in range(2):
                h_out = hpool.tile([128, 16, TOK], BF16, tag="h", name=f"h_l{layer}")
                h8_out = None
                if layer == 0:
                    h8_out = hpool.tile([128, 16, TOK], FP8, tag="h8", name="h8b")
                for j in range(16):
                    wn = wpool.tile([128, 10, 128], BF16, tag="wnh", bufs=3)
                    nc.sync.dma_start(out=wn[:], in_=whw_nl[layer, j])
                    wn8 = wpool.tile([128, 3, 2, 128], FP8, tag="wn8", bufs=3)
                    nc.sync.dma_start(out=wn8[:], in_=whw_n8[layer, j])
                    wg = wpool.tile([128, 8, 2, 128], FP8, tag="wg")
                    nc.sync.dma_start(out=wg[:], in_=whw_g8[layer, j])
                    p_nl = convp.tile([128, 2, TOK], FP32, tag="ps", name="psnl")[:, 0, :]
                    corder = [c for c in (C_ORDER if layer == 0 else range(16)) if c < 10]
                    for ci, c in enumerate(corder):
                        nc.tensor.matmul(
                            out=p_nl[:], lhsT=wn[:, c, :], rhs=h_in[:, c, :],
                            start=(ci == 0), stop=False)
                    for c2 in (5, 6, 7):
                        nc.tensor.matmul(
                            out=p_nl[:], lhsT=wn8[:, c2 - 5],
                            rhs=h8_in[:, 2 * c2:2 * c2 + 2, :],
                            start=False, stop=(c2 == 7),
                            perf_mode=mybir.MatmulPerfMode.DoubleRow)
                    p_g = convp.tile([128, 2, TOK], FP32, tag="ps", name="psg")[:, 0, :]
                    g_order = C2_ORDER if layer == 0 else list(range(8))
                    for gi, c2 in enumerate(g_order):
                        nc.tensor.matmul(
                            out=p_g[:], lhsT=wg[:, c2], rhs=h8_in[:, 2 * c2:2 * c2 + 2, :],
                            start=(gi == 0), stop=(gi == 7),
                            perf_mode=mybir.MatmulPerfMode.DoubleRow)
                    nl = mpool.tile([128, TOK], BF16, tag="nl")
                    gt = mpool.tile([128, TOK], BF16, tag="gt")
                    nc.scalar.activation(
                        out=nl[:], in_=p_nl[:],
                        func=mybir.ActivationFunctionType.Relu,
                        bias=bhw_t[:, layer, j, 0:1], scale=1.0)
                    nc.scalar.activation(
                        out=gt[:], in_=p_g[:],
                        func=mybir.ActivationFunctionType.Sigmoid,
                        bias=bhw_t[:, layer, j, 1:2], scale=1.0)
                    d = mpool.tile([128, TOK], BF16, tag="d")
                    nc.vector.tensor_tensor(
                        out=d[:], in0=h_in[:, j, :], in1=nl[:],
                        op=mybir.AluOpType.subtract)
                    nc.vector.tensor_mul(out=gt[:], in0=gt[:], in1=d[:])
                    nc.vector.tensor_add(out=h_out[:, j, :], in0=nl[:], in1=gt[:])
                    if h8_out is not None:
                        if j >= 13:
                            nc.vector.tensor_scalar_add(
                                out=h8_out[:, j, :], in0=h_out[:, j, :], scalar1=0.0)
                        else:
                            nc.gpsimd.dma_start(out=h8_out[:, j, :], in_=h_out[:, j, :])
                h_in = h_out
                if h8_out is not None:
                    h8_in = h8_out

            # ---- 5. projection, feature-major out
            for j2 in range(4):
                wp = wpool.tile([128, 16, 128], BF16, tag="wn", bufs=2)
                nc.sync.dma_start(out=wp[:], in_=wproj[j2])
                p_o = convp.tile([128, 2, TOK], FP32, tag="ps", name="pso")[:, 0, :]
                for c in range(16):
                    nc.tensor.matmul(
                        out=p_o[:], lhsT=wp[:, c, :], rhs=h_in[:, c, :],
                        start=(c == 0), stop=(c == 15))
                ot = cpool.tile([128, TOK], FP32, tag="ot", name="ot", bufs=2)
                nc.scalar.activation(
                    out=ot[:], in_=p_o[:],
                    func=mybir.ActivationFunctionType.Identity,
                    bias=bproj_t[:, j2:j2 + 1], scale=1.0)
                nc.sync.dma_start(out=out[:, j2, :], in_=ot[:])

    nc.compile()
    return nc


_CACHED = {}


def _prep(inputs):
    """Host-side layout prep: sharding, index packing, weight packing."""
    chars = np.asarray(inputs["chars"]).astype(np.int64).reshape(NTOK, L)
    chars_pad = np.full((NTOK, TP), ZERO_ROW, np.int64)
    chars_pad[:, :L] = chars

    emb = np.asarray(inputs["char_emb"], np.float32)
    table = np.zeros((PAD_V, 128), np.float32)
    table[:CHAR_VOCAB, :CHAR_DIM] = emb
    table = table.astype(ml_dtypes.bfloat16)

    wc = np.zeros((7, CHAR_DIM, N_FILTERS), np.float32)
    off = 0
    for fi, (w, n) in enumerate(FILTERS):
        cw = np.asarray(inputs[f"conv_w_{fi}"], np.float32)  # (n, 16, w)
        wc[:w, :, off:off + n] = cw.transpose(2, 1, 0)
        off += n
    wconv = wc.reshape(KDIM, N_FILTERS).astype(ml_dtypes.bfloat16)
    bconv = np.concatenate([np.asarray(inputs[f"conv_b_{i}"], np.float32)
                            for i in range(7)])
    bconv_dev = bconv.reshape(16, 128).T.copy()  # (128, 16)

    whw_nl = np.zeros((2, 16, 128, 10, 128), np.float32)
    whw_n8 = np.zeros((2, 16, 128, 3, 2, 128), np.float32)
    whw_g8 = np.zeros((2, 16, 128, 8, 2, 128), np.float32)
    bhw = np.zeros((128, 2, 16, 2), np.float32)
    for l in range(2):
        W = np.asarray(inputs[f"hw_w_{l}"], np.float32)   # (4096, 2048)
        bb = np.asarray(inputs[f"hw_b_{l}"], np.float32)  # (4096,)
        WT = W.T  # (ic 2048, oc 4096)
        # nl: (j, p, c, o) = WT[128c+p, 128j+o]; chunks 12-15 go fp8 DR
        nlv = WT[:, 0:2048].reshape(16, 128, 16, 128)       # (c, p, j, o)
        whw_nl[l] = nlv[0:10].transpose(2, 1, 0, 3)         # (j, p, c, o)
        n8v = nlv[10:16].reshape(3, 2, 128, 16, 128)        # (c2, i, p, j, o)
        whw_n8[l] = n8v.transpose(3, 2, 0, 1, 4)            # (j, p, c2, i, o)
        gv = WT[:, 2048:4096].reshape(8, 2, 128, 16, 128)   # (c2, i, p, j, o)
        whw_g8[l] = gv.transpose(3, 2, 0, 1, 4)             # (j, p, c2, i, o)
        bhw[:, l, :, 0] = bb[0:2048].reshape(16, 128).T
        bhw[:, l, :, 1] = bb[2048:4096].reshape(16, 128).T
    whw_nl = whw_nl.astype(ml_dtypes.bfloat16)
    whw_n8 = whw_n8.astype(ml_dtypes.float8_e4m3)
    whw_g8 = whw_g8.astype(ml_dtypes.float8_e4m3)

    Wp = np.asarray(inputs["proj_w"], np.float32)  # (512, 2048)
    WpT = Wp.T  # (2048, 512)
    wproj = WpT.reshape(16, 128, 4, 128).transpose(2, 1, 0, 3).copy()
    wproj = wproj.astype(ml_dtypes.bfloat16)      # (j2, p, c, o)
    bproj = np.asarray(inputs["proj_b"], np.float32).reshape(4, 128).T.copy()

    shared = dict(table=table, wconv=wconv, bconv=bconv_dev, whw_nl=whw_nl, whw_n8=whw_n8,
                  whw_g8=whw_g8, bhw=bhw, wproj=wproj, bproj=bproj)

    in_maps = []
    for core in range(N_CORES):
        cp = chars_pad[core * TOK:(core + 1) * TOK]  # (512, 56)
        idx_flat = cp.T.reshape(-1).astype(np.int16)  # j = t'*512 + n
        idx16 = idx_flat.reshape(NI // 16, 16).T.copy()  # (16, NI/16)
        idx16 = np.tile(idx16, (8, 1))  # (128, NI/16)
        m = dict(shared)
        m["idx"] = idx16
        in_maps.append(m)
    return in_maps


def kernel(**inputs) -> np.ndarray:
    if "nc" not in _CACHED:
        _CACHED["nc"] = build_module()
    nc = _CACHED["nc"]
    in_maps = _prep(inputs)
    res = run_bass_kernel_spmd(nc, in_maps, core_ids=list(range(N_CORES)))
    # out[p, j2, n] -> full[n, 128*j2 + p]
    parts = []
    for r in res.results:
        o = r["out"]  # (128, 4, 512)
        parts.append(o.transpose(2, 1, 0).reshape(TOK, PROJ_DIM))
    full = np.concatenate(parts, axis=0)
    return full.reshape(B, S, PROJ_DIM)


if __name__ == "__main__":
    order, load = CONV_SCHED, CONV_LOAD
    from collections import Counter
    print("rounds:", len(order), Counter(t for (_, _, _, t, _) in order))
    print("loads (us):", {k: round(v / 1000, 1) for k, v in load.items()})


# revision 17
# speedup vs baseline: 1.0134x; 1.0007x over previous
"""CharCNN token embedder (ELMo-style) on 8 Trainium2 NeuronCores — v2.

Data-parallel over 4096 = 16*256 tokens (512 per core), weights replicated.

Per-core pipeline (all phases overlap via tile-framework semaphores):
  1. Chunked dma_gather (8 x 3584 idx) pulls char-embedding rows into
     feature-major chunks xg_r[d, (t', n)]; per-chunk shifted copies build
     the im2col patch matrix xs[(k,d), (t, n)] incrementally, so conv
     matmuls start while later chunks are still in flight.
  2. Conv = matmuls with K=112 per 128-channel tile, one per output
     position (rounds of <=4 positions into a [128,4,512] PSUM group,
     double-buffered).  Position max-pool runs as a statically scheduled
     mix of drain types balanced across engines:
       X: DVE folds PSUM directly into a bf16 acc pair
       Z: ACT copies PSUM->bf16, DVE folds
       W: ACT copies PSUM->bf16, GPSIMD(Pool) folds (Pool cannot read PSUM)
  3. Per-tile finale: combine accs, relu+bias -> h1 (bf16) + h8 (fp8e4).
  4. 2 highway layers: nonlinear half in bf16 (16 K-chunks), gate half in
     fp8e4 DoubleRow (8 chunk-pairs, 2x PE throughput; sigmoid gating makes
     gate-half quantization error negligible).  ACT does relu/sigmoid,
     DVE does the gating arithmetic.
  5. Projection to 512 in bf16; output stored feature-major [128, 4, 512]
     fp32 and transposed on the host.
"""

import numpy as np
import ml_dtypes

import concourse.bass as bass
import concourse.mybir as mybir
import concourse.tile as tile
from concourse import bacc
from concourse.bass_utils import run_bass_kernel_spmd
from concourse.vector_clock import ScopedClock

# ---------------------------------------------------------------- constants
B, S, L = 16, 256, 50
CHAR_DIM = 16
CHAR_VOCAB = 262
PAD_V = 264
ZERO_ROW = 262
FILTERS = [(1, 32), (2, 32), (3, 64), (4, 128), (5, 256), (6, 512), (7, 1024)]
N_FILTERS = 2048
PROJ_DIM = 512
N_CORES = 8
NTOK = B * S
TOK = NTOK // N_CORES        # 512 tokens per core
TP = 56                      # padded positions per token (50 + 6)
NI = TOK * TP                # gather indices per core = 28672
# gather chunk boundaries in t'-positions: small head chunk so conv starts
# early, small tail so xg tiles stay <= 3584 indices
CHUNK_BOUNDS = [(0, 8), (8, 15), (15, 22), (22, 29),
                (29, 36), (36, 43), (43, 50), (50, 56)]
NPOS = 50
FREE = TOK * NPOS            # 25600
KDIM = 112

# per-tile valid position count (tile 0 = w1/w2/w3 mixed; see tail handling)
TILE_TCNT = [50, 47, 46, 46, 45, 45, 45, 45, 44, 44, 44, 44, 44, 44, 44, 44]

BF16 = mybir.dt.bfloat16
FP32 = mybir.dt.float32
FP8 = mybir.dt.float8e4

_MAX_WAITS_PER_INST = 1


def _patched_drain_and_barrier(self, tick_clock, wait_clock):
    # The walrus build in this container rejects CTRL instructions carrying
    # more than one sem wait; spread the kernel-tail drain waits over NOPs.
    nc = self.nc
    carrier = nc.sync.nop()
    wait_clock.add_sem_waits(carrier.ins, ScopedClock({None: tick_clock.global_clock}))
    si = carrier.ins.sync_info
    waits = list(si.on_wait) if si is not None and si.on_wait else []
    if len(waits) > _MAX_WAITS_PER_INST:
        carrier.ins.sync_info = mybir.SyncInfo(
            on_wait=waits[:_MAX_WAITS_PER_INST],
            on_update=list(si.on_update) if si.on_update else [])
        for i in range(_MAX_WAITS_PER_INST, len(waits), _MAX_WAITS_PER_INST):
            extra = nc.sync.nop()
            extra.ins.sync_info = mybir.SyncInfo(
                on_wait=waits[i:i + _MAX_WAITS_PER_INST], on_update=[])
    nc.sync.drain()
    nc.all_engine_barrier()
    assert self.sems is not None
    popped = nc._tile_sem_poison_stack.pop()
    assert popped is self._sem_poison
    nc.clear_and_free_semaphores(list(self.sems.allocated().values()))
    nc.all_engine_barrier()


tile.TileContext._drain_and_barrier = _patched_drain_and_barrier


# ------------------------------------------------------- static drain plan
def build_conv_schedule():
    """Greedy X/Z assignment of conv pooling rounds (Pool engine cannot run
    elementwise ops on TRN2, so only DVE folds PSUM directly (X) or folds
    ACT-staged bf16 copies (Z)).  Finale relu+bias runs on DVE via 4x-mode
    tensor_scalar; h8 casts ride on gpsimd cast-DMAs."""
    cost = {
        "X": {2: [("DVE", 1192)], 1: [("DVE", 658)]},
        "Z": {2: [("ACT", 1038), ("DVE", 594)], 1: [("ACT", 612), ("DVE", 387)]},
    }
    load = {"DVE": 9000.0, "ACT": 9000.0}
    fold_eng = {"X": "DVE", "Z": "DVE"}
    cursors = [0] * 16
    order = []
    while True:
        alive = [i for i in range(16) if cursors[i] < TILE_TCNT[i]]
        if not alive:
            break
        alive.sort(key=lambda i: -(TILE_TCNT[i] - cursors[i]))
        for i in alive:
            tcnt = TILE_TCNT[i]
            t0 = cursors[i]
            if i == 0 and t0 == 48:
                order.append((i, t0, 2, "T0TAIL", False))
                load["DVE"] += 1316
                cursors[i] = tcnt
            else:
                lim = 48 if i == 0 else tcnt
                nt = min(2, lim - t0)
                if t0 == 0:
                    # tile init: direct copy into the acc (no fold) on the
                    # less-loaded PSUM-capable engine
                    if load["ACT"] + 1038 < load["DVE"] + 1192:
                        order.append((i, t0, nt, "Z0", True))
                        load["ACT"] += 1038
                    else:
                        order.append((i, t0, nt, "X", True))
                        load["DVE"] += 1192
                    cursors[i] = nt
                    continue
                best, bestkey = None, None
                for typ in ("X", "Z"):
                    m = max(max(load.values()),
                            *[load[e] + c for e, c in cost[typ][nt]])
                    key = (m, load["DVE"] + dict(cost[typ][nt]).get("DVE", 0.0))
                    if bestkey is None or key < bestkey:
                        best, bestkey = typ, key
                order.append((i, t0, nt, best, t0 == 0))
                for e, c in cost[best][nt]:
                    load[e] += c
                cursors[i] = t0 + nt
            if cursors[i] >= TILE_TCNT[i]:
                load["DVE"] += 520  # m1 max + tensor_scalar relu+bias
    return order, load


CONV_SCHED, CONV_LOAD = build_conv_schedule()

def conv_completion_order(sched):
    seen = []
    for (i, t0, nt, typ, first) in sched:
        done = (t0 + nt >= TILE_TCNT[i]) or typ == "T0TAIL"
        if done:
            seen.append(i)
    return seen

C_ORDER = conv_completion_order(CONV_SCHED)
_POS = {c: k for k, c in enumerate(C_ORDER)}
C2_ORDER = sorted(range(8), key=lambda c2: max(_POS[2 * c2], _POS[2 * c2 + 1]))
# last-completing tiles: their h8 cast gates the highway start
LATE_TILES = set(C_ORDER[-3:])



# ---------------------------------------------------------------- device IR
def build_module():
    nc = bacc.Bacc()
    SIdx = NI // 16  # 1792 int16 columns

    table = nc.dram_tensor("table", [PAD_V, 128], BF16, kind="ExternalInput")
    idx = nc.dram_tensor("idx", [128, SIdx], mybir.dt.int16, kind="ExternalInput")
    wconv = nc.dram_tensor("wconv", [KDIM, N_FILTERS], BF16, kind="ExternalInput")
    bconv = nc.dram_tensor("bconv", [128, 16], FP32, kind="ExternalInput")
    whw_nl = nc.dram_tensor("whw_nl", [2, 16, 128, 10, 128], BF16, kind="ExternalInput")
    whw_n8 = nc.dram_tensor("whw_n8", [2, 16, 128, 3, 2, 128], FP8, kind="ExternalInput")
    whw_g8 = nc.dram_tensor("whw_g8", [2, 16, 128, 8, 2, 128], FP8, kind="ExternalInput")
    bhw = nc.dram_tensor("bhw", [128, 2, 16, 2], FP32, kind="ExternalInput")
    wproj = nc.dram_tensor("wproj", [4, 128, 16, 128], BF16, kind="ExternalInput")
    bproj = nc.dram_tensor("bproj", [128, 4], FP32, kind="ExternalInput")
    out = nc.dram_tensor("out", [128, 4, TOK], FP32, kind="ExternalOutput")

    with tile.TileContext(nc) as tc:
        with (
            tc.tile_pool(name="consts", bufs=1) as cpool,
            tc.tile_pool(name="gather", bufs=2) as gpool,
            tc.tile_pool(name="xs", bufs=1) as xspool,
            tc.tile_pool(name="accs", bufs=1) as apool,
            tc.tile_pool(name="stage", bufs=4) as spool,
            tc.tile_pool(name="hbuf", bufs=2) as hpool,
            tc.tile_pool(name="wstream", bufs=2) as wpool,
            tc.tile_pool(name="small", bufs=2) as mpool,
            tc.tile_pool(name="convp", bufs=4, space="PSUM") as convp,
        ):
            # ---- constants in
            idx_t = cpool.tile([128, SIdx], mybir.dt.int16)
            nc.sync.dma_start(out=idx_t[:, 0:256], in_=idx[:, 0:256])
            nc.sync.dma_start(out=idx_t[:, 256:], in_=idx[:, 256:])
            wconv_t = cpool.tile([KDIM, N_FILTERS], BF16)
            nc.sync.dma_start(out=wconv_t[:], in_=wconv[:])
            bconv_t = cpool.tile([128, 16], FP32)
            nc.sync.dma_start(out=bconv_t[:], in_=bconv[:])
            bhw_t = cpool.tile([128, 2, 16, 2], FP32)
            nc.sync.dma_start(out=bhw_t[:], in_=bhw[:])
            bproj_t = cpool.tile([128, 4], FP32)
            nc.sync.dma_start(out=bproj_t[:], in_=bproj[:])

            # ---- 1. chunked gather + incremental im2col build
            # PE p-state warmup: dummy matmuls right after wconv lands so the
            # ramp clock has matured before the first real conv matmul
            Pw = convp.tile([128, 2, TOK], FP32, tag="ps", name="warm")
            for wi in range(6):
                nc.tensor.matmul(
                    out=Pw[:, wi % 2, :], lhsT=wconv_t[:, 0:128],
                    rhs=wconv_t[:, 512:1024], start=True, stop=True)
            xs = xspool.tile([KDIM, FREE], BF16)
            for (p_lo, p_hi) in CHUNK_BOUNDS:
                n_idx = (p_hi - p_lo) * TOK
                xgc = gpool.tile([128, 1, 4096], BF16, tag="xg")
                nc.gpsimd.dma_gather(
                    out_ap=xgc[:, :, 0:n_idx],
                    in_ap=table[:],
                    idxs_ap=idx_t[:, p_lo * 32:p_hi * 32],
                    num_idxs=n_idx,
                    num_idxs_reg=n_idx,
                    elem_size=128,
                    transpose=True,
                    single_packet=False,
                )
                piece_eng = ([nc.sync, nc.scalar] if p_lo == 0
                             else [nc.sync])
                for k in range(7):
                    t_lo = max(0, p_lo - k)
                    t_hi = min(NPOS, p_hi - k)
                    if t_lo >= t_hi:
                        continue
                    s0 = t_lo + k - p_lo
                    piece_eng[k % len(piece_eng)].dma_start(
                        out=xs[16 * k:16 * (k + 1), TOK * t_lo:TOK * t_hi],
                        in_=xgc[0:16, 0, TOK * s0:TOK * (s0 + (t_hi - t_lo))],
                    )

            # ---- 2+3. conv rounds (static schedule) + finales
            h1 = hpool.tile([128, 16, TOK], BF16, tag="h")
            h8a = hpool.tile([128, 16, TOK], FP8, tag="h8")
            accd = {}
            accp = {}
            accp_used = set()
            n_done = 0
            h8_on_dve = {}
            for (i, t0, nt, typ, first) in CONV_SCHED:
                lhsT = wconv_t[:, 128 * i:128 * (i + 1)]
                P = convp.tile([128, 2, TOK], FP32, tag="ps")
                for rpos in range(nt):
                    t = t0 + rpos
                    nc.tensor.matmul(
                        out=P[:, rpos, :], lhsT=lhsT,
                        rhs=xs[:, TOK * t:TOK * (t + 1)],
                        start=True, stop=True,
                    )
                if first:
                    accd[i] = apool.tile([128, 2, TOK], BF16, tag=f"accd{i}", name=f"accd{i}")
                ad = accd[i]
                if typ == "Z0":
                    nc.scalar.copy(out=ad[:, 0:nt, :], in_=P[:, 0:nt, :])
                elif typ == "T0TAIL":
                    # pos t0 valid rows 0:64 (w1+w2), pos t0+1 rows 0:32 (w1)
                    nc.vector.tensor_tensor(
                        out=ad[0:64, 0:1, :], in0=ad[0:64, 0:1, :],
                        in1=P[0:64, 0:1, :], op=mybir.AluOpType.max)
                    nc.vector.tensor_tensor(
                        out=ad[0:32, 1:2, :], in0=ad[0:32, 1:2, :],
                        in1=P[0:32, 1:2, :], op=mybir.AluOpType.max)
                elif typ == "X":
                    if first:
                        nc.vector.tensor_scalar_add(
                            out=ad[:, 0:nt, :], in0=P[:, 0:nt, :], scalar1=0.0)
                    else:
                        nc.vector.tensor_tensor(
                            out=ad[:, 0:nt, :], in0=ad[:, 0:nt, :],
                            in1=P[:, 0:nt, :], op=mybir.AluOpType.max)
                else:
                    s4 = spool.tile([128, 2, TOK], BF16, tag="s4")
                    nc.scalar.copy(out=s4[:, 0:nt, :], in_=P[:, 0:nt, :])
                    if typ == "Z":
                        if first:
                            nc.vector.tensor_scalar_add(
                                out=ad[:, 0:nt, :], in0=s4[:, 0:nt, :], scalar1=0.0)
                        else:
                            nc.vector.tensor_tensor(
                                out=ad[:, 0:nt, :], in0=ad[:, 0:nt, :],
                                in1=s4[:, 0:nt, :], op=mybir.AluOpType.max)
                    else:  # W: pool folds
                        if i not in accp:
                            accp[i] = apool.tile([128, 2, TOK], BF16, tag=f"accp{i}", name=f"accp{i}")
                            nc.gpsimd.tensor_scalar_add(
                                out=accp[i][:, 0:nt, :], in0=s4[:, 0:nt, :], scalar1=0.0)
                        else:
                            ap_ = accp[i]
                            nc.gpsimd.scalar_tensor_tensor(
                                out=ap_[:, 0:nt, :], in0=s4[:, 0:nt, :], scalar=0.0,
                                in1=ap_[:, 0:nt, :],
                                op0=mybir.AluOpType.add, op1=mybir.AluOpType.max)
                        accp_used.add(i)
                # finale when tile complete
                done = (t0 + nt >= TILE_TCNT[i]) or (typ == "T0TAIL")
                if done:
                    if i in accp:
                        nc.vector.tensor_tensor(
                            out=ad[:], in0=ad[:], in1=accp[i][:],
                            op=mybir.AluOpType.max)
                    m1 = mpool.tile([128, TOK], BF16, tag="m1")
                    nc.vector.tensor_tensor(
                        out=m1[:], in0=ad[:, 0, :], in1=ad[:, 1, :],
                        op=mybir.AluOpType.max)
                    nc.vector.tensor_scalar(
                        out=h1[:, i, :], in0=m1[:],
                        scalar1=bconv_t[:, i:i + 1], scalar2=0.0,
                        op0=mybir.AluOpType.add, op1=mybir.AluOpType.max)
                    nc.gpsimd.dma_start(out=h8a[:, i, :], in_=h1[:, i, :])
                    n_done += 1

            # ---- 4. highway layers
            h_in, h8_in = h1, h8a
            for layer in range(2):
                h_out = hpool.tile([128, 16, TOK], BF16, tag="h", name=f"h_l{layer}")
                h8_out = None
                if layer == 0:
                    h8_out = hpool.tile([128, 16, TOK], FP8, tag="h8", name="h8b")
                for j in range(16):
                    wn = wpool.tile([128, 10, 128], BF16, tag="wnh", bufs=3)
                    nc.sync.dma_start(out=wn[:], in_=whw_nl[layer, j])
                    wn8 = wpool.tile([128, 3, 2, 128], FP8, tag="wn8", bufs=3)
                    nc.sync.dma_start(out=wn8[:], in_=whw_n8[layer, j])
                    wg = wpool.tile([128, 8, 2, 128], FP8, tag="wg")
                    nc.sync.dma_start(out=wg[:], in_=whw_g8[layer, j])
                    p_nl = convp.tile([128, 2, TOK], FP32, tag="ps", name="psnl")[:, 0, :]
                    corder = [c for c in (C_ORDER if layer == 0 else range(16)) if c < 10]
                    for ci, c in enumerate(corder):
                        nc.tensor.matmul(
                            out=p_nl[:], lhsT=wn[:, c, :], rhs=h_in[:, c, :],
                            start=(ci == 0), stop=False)
                    for c2 in (5, 6, 7):
                        nc.tensor.matmul(
                            out=p_nl[:], lhsT=wn8[:, c2 - 5],
                            rhs=h8_in[:, 2 * c2:2 * c2 + 2, :],
                            start=False, stop=(c2 == 7),
                            perf_mode=mybir.MatmulPerfMode.DoubleRow)
                    p_g = convp.tile([128, 2, TOK], FP32, tag="ps", name="psg")[:, 0, :]
                    g_order = C2_ORDER if layer == 0 else list(range(8))
                    for gi, c2 in enumerate(g_order):
                        nc.tensor.matmul(
                            out=p_g[:], lhsT=wg[:, c2], rhs=h8_in[:, 2 * c2:2 * c2 + 2, :],
                            start=(gi == 0), stop=(gi == 7),
                            perf_mode=mybir.MatmulPerfMode.DoubleRow)
                    nl = mpool.tile([128, TOK], BF16, tag="nl")
                    gt = mpool.tile([128, TOK], BF16, tag="gt")
                    nc.scalar.activation(
                        out=nl[:], in_=p_nl[:],
                        func=mybir.ActivationFunctionType.Relu,
                        bias=bhw_t[:, layer, j, 0:1], scale=1.0)
                    nc.scalar.activation(
                        out=gt[:], in_=p_g[:],
                        func=mybir.ActivationFunctionType.Sigmoid,
                        bias=bhw_t[:, layer, j, 1:2], scale=1.0)
                    d = mpool.tile([128, TOK], BF16, tag="d")
                    nc.vector.tensor_tensor(
                        out=d[:], in0=h_in[:, j, :], in1=nl[:],
                        op=mybir.AluOpType.subtract)
                    nc.vector.tensor_mul(out=gt[:], in0=gt[:], in1=d[:])
                    nc.vector.tensor_add(out=h_out[:, j, :], in0=nl[:], in1=gt[:])
                    if h8_out is not None:
                        if j >= 13:
                            nc.vector.tensor_scalar_add(
                                out=h8_out[:, j, :], in0=h_out[:, j, :], scalar1=0.0)
                        else:
                            nc.gpsimd.dma_start(out=h8_out[:, j, :], in_=h_out[:, j, :])
                h_in = h_out
                if h8_out is not None:
                    h8_in = h8_out

            # ---- 5. projection, feature-major out
            for j2 in range(4):
                wp = wpool.tile([128, 16, 128], BF16, tag="wn", bufs=3)
                nc.sync.dma_start(out=wp[:], in_=wproj[j2])
                p_o = convp.tile([128, 2, TOK], FP32, tag="ps", name="pso")[:, 0, :]
                for c in range(16):
                    nc.tensor.matmul(
                        out=p_o[:], lhsT=wp[:, c, :], rhs=h_in[:, c, :],
                        start=(c == 0), stop=(c == 15))
                ot = cpool.tile([128, TOK], FP32, tag="ot", name="ot", bufs=2)
                nc.scalar.activation(
                    out=ot[:], in_=p_o[:],
                    func=mybir.ActivationFunctionType.Identity,
                    bias=bproj_t[:, j2:j2 + 1], scale=1.0)
                nc.sync.dma_start(out=out[:, j2, :], in_=ot[:])

    nc.compile()
    return nc


_CACHED = {}


def _prep(inputs):
    """Host-side layout prep: sharding, index packing, weight packing."""
    chars = np.asarray(inputs["chars"]).astype(np.int64).reshape(NTOK, L)
    chars_pad = np.full((NTOK, TP), ZERO_ROW, np.int64)
    chars_pad[:, :L] = chars

    emb = np.asarray(inputs["char_emb"], np.float32)
    table = np.zeros((PAD_V, 128), np.float32)
    table[:CHAR_VOCAB, :CHAR_DIM] = emb
    table = table.astype(ml_dtypes.bfloat16)

    wc = np.zeros((7, CHAR_DIM, N_FILTERS), np.float32)
    off = 0
    for fi, (w, n) in enumerate(FILTERS):
        cw = np.asarray(inputs[f"conv_w_{fi}"], np.float32)  # (n, 16, w)
        wc[:w, :, off:off + n] = cw.transpose(2, 1, 0)
        off += n
    wconv = wc.reshape(KDIM, N_FILTERS).astype(ml_dtypes.bfloat16)
    bconv = np.concatenate([np.asarray(inputs[f"conv_b_{i}"], np.float32)
                            for i in range(7)])
    bconv_dev = bconv.reshape(16, 128).T.copy()  # (128, 16)

    whw_nl = np.zeros((2, 16, 128, 10, 128), np.float32)
    whw_n8 = np.zeros((2, 16, 128, 3, 2, 128), np.float32)
    whw_g8 = np.zeros((2, 16, 128, 8, 2, 128), np.float32)
    bhw = np.zeros((128, 2, 16, 2), np.float32)
    for l in range(2):
        W = np.asarray(inputs[f"hw_w_{l}"], np.float32)   # (4096, 2048)
        bb = np.asarray(inputs[f"hw_b_{l}"], np.float32)  # (4096,)
        WT = W.T  # (ic 2048, oc 4096)
        # nl: (j, p, c, o) = WT[128c+p, 128j+o]; chunks 12-15 go fp8 DR
        nlv = WT[:, 0:2048].reshape(16, 128, 16, 128)       # (c, p, j, o)
        whw_nl[l] = nlv[0:10].transpose(2, 1, 0, 3)         # (j, p, c, o)
        n8v = nlv[10:16].reshape(3, 2, 128, 16, 128)        # (c2, i, p, j, o)
        whw_n8[l] = n8v.transpose(3, 2, 0, 1, 4)            # (j, p, c2, i, o)
        gv = WT[:, 2048:4096].reshape(8, 2, 128, 16, 128)   # (c2, i, p, j, o)
        whw_g8[l] = gv.transpose(3, 2, 0, 1, 4)             # (j, p, c2, i, o)
        bhw[:, l, :, 0] = bb[0:2048].reshape(16, 128).T
        bhw[:, l, :, 1] = bb[2048:4096].reshape(16, 128).T
    whw_nl = whw_nl.astype(ml_dtypes.bfloat16)
    whw_n8 = whw_n8.astype(ml_dtypes.float8_e4m3)
    whw_g8 = whw_g8.astype(ml_dtypes.float8_e4m3)

    Wp = np.asarray(inputs["proj_w"], np.float32)  # (512, 2048)
    WpT = Wp.T  # (2048, 512)
    wproj = WpT.reshape(16, 128, 4, 128).transpose(2, 1, 0, 3).copy()
    wproj = wproj.astype(ml_dtypes.bfloat16)      # (j2, p, c, o)
    bproj = np.asarray(inputs["proj_b"], np.float32).reshape(4, 128).T.copy()

    shared = dict(table=table, wconv=wconv, bconv=bconv_dev, whw_nl=whw_nl, whw_n8=whw_n8,
                  whw_g8=whw_g8, bhw=bhw, wproj=wproj, bproj=bproj)

    in_maps = []
    for core in range(N_CORES):
        cp = chars_pad[core * TOK:(core + 1) * TOK]  # (512, 56)
        idx_flat = cp.T.reshape(-1).astype(np.int16)  # j = t'*512 + n
        idx16 = idx_flat.reshape(NI // 16, 16).T.copy()  # (16, NI/16)
        idx16 = np.tile(idx16, (8, 1))  # (128, NI/16)
        m = dict(shared)
        m["idx"] = idx16
        in_maps.append(m)
    return in_maps


def kernel(**inputs) -> np.ndarray:
    if "nc" not in _CACHED:
        _CACHED["nc"] = build_module()
    nc = _CACHED["nc"]
    in_maps = _prep(inputs)
    res = run_bass_kernel_spmd(nc, in_maps, core_ids=list(range(N_CORES)))
    # out[p, j2, n] -> full[n, 128*j2 + p]
    parts = []
    for r in res.results:
        o = r["out"]  # (128, 4, 512)
        parts.append(o.transpose(2, 1, 0).reshape(TOK, PROJ_DIM))
    full = np.concatenate(parts, axis=0)
    return full.reshape(B, S, PROJ_DIM)


if __name__ == "__main__":
    order, load = CONV_SCHED, CONV_LOAD
    from collections import Counter
    print("rounds:", len(order), Counter(t for (_, _, _, t, _) in order))
    print("loads (us):", {k: round(v / 1000, 1) for k, v in load.items()})


# revision 18
# speedup vs baseline: 1.0151x; 1.0017x over previous
"""CharCNN token embedder (ELMo-style) on 8 Trainium2 NeuronCores — v2.

Data-parallel over 4096 = 16*256 tokens (512 per core), weights replicated.

Per-core pipeline (all phases overlap via tile-framework semaphores):
  1. Chunked dma_gather (8 x 3584 idx) pulls char-embedding rows into
     feature-major chunks xg_r[d, (t', n)]; per-chunk shifted copies build
     the im2col patch matrix xs[(k,d), (t, n)] incrementally, so conv
     matmuls start while later chunks are still in flight.
  2. Conv = matmuls with K=112 per 128-channel tile, one per output
     position (rounds of <=4 positions into a [128,4,512] PSUM group,
     double-buffered).  Position max-pool runs as a statically scheduled
     mix of drain types balanced across engines:
       X: DVE folds PSUM directly into a bf16 acc pair
       Z: ACT copies PSUM->bf16, DVE folds
       W: ACT copies PSUM->bf16, GPSIMD(Pool) folds (Pool cannot read PSUM)
  3. Per-tile finale: combine accs, relu+bias -> h1 (bf16) + h8 (fp8e4).
  4. 2 highway layers: nonlinear half in bf16 (16 K-chunks), gate half in
     fp8e4 DoubleRow (8 chunk-pairs, 2x PE throughput; sigmoid gating makes
     gate-half quantization error negligible).  ACT does relu/sigmoid,
     DVE does the gating arithmetic.
  5. Projection to 512 in bf16; output stored feature-major [128, 4, 512]
     fp32 and transposed on the host.
"""

import numpy as np
import ml_dtypes

import concourse.bass as bass
import concourse.mybir as mybir
import concourse.tile as tile
from concourse import bacc
from concourse.bass_utils import run_bass_kernel_spmd
from concourse.vector_clock import ScopedClock

# ---------------------------------------------------------------- constants
B, S, L = 16, 256, 50
CHAR_DIM = 16
CHAR_VOCAB = 262
PAD_V = 264
ZERO_ROW = 262
FILTERS = [(1, 32), (2, 32), (3, 64), (4, 128), (5, 256), (6, 512), (7, 1024)]
N_FILTERS = 2048
PROJ_DIM = 512
N_CORES = 8
NTOK = B * S
TOK = NTOK // N_CORES        # 512 tokens per core
TP = 56                      # padded positions per token (50 + 6)
NI = TOK * TP                # gather indices per core = 28672
# gather chunk boundaries in t'-positions: small head chunk so conv starts
# early, small tail so xg tiles stay <= 3584 indices
CHUNK_BOUNDS = [(0, 8), (8, 14), (14, 21), (21, 28),
                (28, 35), (35, 42), (42, 49), (49, 56)]
NPOS = 50
FREE = TOK * NPOS            # 25600
KDIM = 112

# per-tile valid position count (tile 0 = w1/w2/w3 mixed; see tail handling)
TILE_TCNT = [50, 47, 46, 46, 45, 45, 45, 45, 44, 44, 44, 44, 44, 44, 44, 44]

BF16 = mybir.dt.bfloat16
FP32 = mybir.dt.float32
FP8 = mybir.dt.float8e4

_MAX_WAITS_PER_INST = 1


def _patched_drain_and_barrier(self, tick_clock, wait_clock):
    # The walrus build in this container rejects CTRL instructions carrying
    # more than one sem wait; spread the kernel-tail drain waits over NOPs.
    nc = self.nc
    carrier = nc.sync.nop()
    wait_clock.add_sem_waits(carrier.ins, ScopedClock({None: tick_clock.global_clock}))
    si = carrier.ins.sync_info
    waits = list(si.on_wait) if si is not None and si.on_wait else []
    if len(waits) > _MAX_WAITS_PER_INST:
        carrier.ins.sync_info = mybir.SyncInfo(
            on_wait=waits[:_MAX_WAITS_PER_INST],
            on_update=list(si.on_update) if si.on_update else [])
        for i in range(_MAX_WAITS_PER_INST, len(waits), _MAX_WAITS_PER_INST):
            extra = nc.sync.nop()
            extra.ins.sync_info = mybir.SyncInfo(
                on_wait=waits[i:i + _MAX_WAITS_PER_INST], on_update=[])
    nc.sync.drain()
    nc.all_engine_barrier()
    assert self.sems is not None
    popped = nc._tile_sem_poison_stack.pop()
    assert popped is self._sem_poison
    nc.clear_and_free_semaphores(list(self.sems.allocated().values()))
    nc.all_engine_barrier()


tile.TileContext._drain_and_barrier = _patched_drain_and_barrier


# ------------------------------------------------------- static drain plan
def build_conv_schedule():
    """Greedy X/Z assignment of conv pooling rounds (Pool engine cannot run
    elementwise ops on TRN2, so only DVE folds PSUM directly (X) or folds
    ACT-staged bf16 copies (Z)).  Finale relu+bias runs on DVE via 4x-mode
    tensor_scalar; h8 casts ride on gpsimd cast-DMAs."""
    cost = {
        "X": {2: [("DVE", 1192)], 1: [("DVE", 658)]},
        "Z": {2: [("ACT", 1038), ("DVE", 594)], 1: [("ACT", 612), ("DVE", 387)]},
    }
    load = {"DVE": 9000.0, "ACT": 9000.0}
    fold_eng = {"X": "DVE", "Z": "DVE"}
    cursors = [0] * 16
    order = []
    while True:
        alive = [i for i in range(16) if cursors[i] < TILE_TCNT[i]]
        if not alive:
            break
        alive.sort(key=lambda i: -(TILE_TCNT[i] - cursors[i]))
        for i in alive:
            tcnt = TILE_TCNT[i]
            t0 = cursors[i]
            if i == 0 and t0 == 48:
                order.append((i, t0, 2, "T0TAIL", False))
                load["DVE"] += 1316
                cursors[i] = tcnt
            else:
                lim = 48 if i == 0 else tcnt
                nt = min(2, lim - t0)
                if t0 == 0:
                    # tile init: direct copy into the acc (no fold) on the
                    # less-loaded PSUM-capable engine
                    if load["ACT"] + 1038 < load["DVE"] + 1192:
                        order.append((i, t0, nt, "Z0", True))
                        load["ACT"] += 1038
                    else:
                        order.append((i, t0, nt, "X", True))
                        load["DVE"] += 1192
                    cursors[i] = nt
                    continue
                best, bestkey = None, None
                for typ in ("X", "Z"):
                    m = max(max(load.values()),
                            *[load[e] + c for e, c in cost[typ][nt]])
                    key = (m, load["DVE"] + dict(cost[typ][nt]).get("DVE", 0.0))
                    if bestkey is None or key < bestkey:
                        best, bestkey = typ, key
                order.append((i, t0, nt, best, t0 == 0))
                for e, c in cost[best][nt]:
                    load[e] += c
                cursors[i] = t0 + nt
            if cursors[i] >= TILE_TCNT[i]:
                load["DVE"] += 520  # m1 max + tensor_scalar relu+bias
    return order, load


CONV_SCHED, CONV_LOAD = build_conv_schedule()

def conv_completion_order(sched):
    seen = []
    for (i, t0, nt, typ, first) in sched:
        done = (t0 + nt >= TILE_TCNT[i]) or typ == "T0TAIL"
        if done:
            seen.append(i)
    return seen

C_ORDER = conv_completion_order(CONV_SCHED)
_POS = {c: k for k, c in enumerate(C_ORDER)}
C2_ORDER = sorted(range(8), key=lambda c2: max(_POS[2 * c2], _POS[2 * c2 + 1]))
# last-completing tiles: their h8 cast gates the highway start
LATE_TILES = set(C_ORDER[-3:])



# ---------------------------------------------------------------- device IR
def build_module():
    nc = bacc.Bacc()
    SIdx = NI // 16  # 1792 int16 columns

    table = nc.dram_tensor("table", [PAD_V, 128], BF16, kind="ExternalInput")
    idx = nc.dram_tensor("idx", [128, SIdx], mybir.dt.int16, kind="ExternalInput")
    wconv = nc.dram_tensor("wconv", [KDIM, N_FILTERS], BF16, kind="ExternalInput")
    bconv = nc.dram_tensor("bconv", [128, 16], FP32, kind="ExternalInput")
    whw_nl = nc.dram_tensor("whw_nl", [2, 16, 128, 10, 128], BF16, kind="ExternalInput")
    whw_n8 = nc.dram_tensor("whw_n8", [2, 16, 128, 3, 2, 128], FP8, kind="ExternalInput")
    whw_g8 = nc.dram_tensor("whw_g8", [2, 16, 128, 8, 2, 128], FP8, kind="ExternalInput")
    bhw = nc.dram_tensor("bhw", [128, 2, 16, 2], FP32, kind="ExternalInput")
    wproj = nc.dram_tensor("wproj", [4, 128, 16, 128], BF16, kind="ExternalInput")
    bproj = nc.dram_tensor("bproj", [128, 4], FP32, kind="ExternalInput")
    out = nc.dram_tensor("out", [128, 4, TOK], FP32, kind="ExternalOutput")

    with tile.TileContext(nc) as tc:
        with (
            tc.tile_pool(name="consts", bufs=1) as cpool,
            tc.tile_pool(name="gather", bufs=2) as gpool,
            tc.tile_pool(name="xs", bufs=1) as xspool,
            tc.tile_pool(name="accs", bufs=1) as apool,
            tc.tile_pool(name="stage", bufs=4) as spool,
            tc.tile_pool(name="hbuf", bufs=2) as hpool,
            tc.tile_pool(name="wstream", bufs=2) as wpool,
            tc.tile_pool(name="small", bufs=2) as mpool,
            tc.tile_pool(name="convp", bufs=4, space="PSUM") as convp,
        ):
            # ---- constants in
            idx_t = cpool.tile([128, SIdx], mybir.dt.int16)
            nc.sync.dma_start(out=idx_t[:, 0:256], in_=idx[:, 0:256])
            nc.sync.dma_start(out=idx_t[:, 256:], in_=idx[:, 256:])
            wconv_t = cpool.tile([KDIM, N_FILTERS], BF16)
            nc.sync.dma_start(out=wconv_t[:], in_=wconv[:])
            bconv_t = cpool.tile([128, 16], FP32)
            nc.sync.dma_start(out=bconv_t[:], in_=bconv[:])
            bhw_t = cpool.tile([128, 2, 16, 2], FP32)
            nc.sync.dma_start(out=bhw_t[:], in_=bhw[:])
            bproj_t = cpool.tile([128, 4], FP32)
            nc.sync.dma_start(out=bproj_t[:], in_=bproj[:])

            # ---- 1. chunked gather + incremental im2col build
            # PE p-state warmup: dummy matmuls right after wconv lands so the
            # ramp clock has matured before the first real conv matmul
            Pw = convp.tile([128, 2, TOK], FP32, tag="ps", name="warm")
            for wi in range(6):
                nc.tensor.matmul(
                    out=Pw[:, wi % 2, :], lhsT=wconv_t[:, 0:128],
                    rhs=wconv_t[:, 512:1024], start=True, stop=True)
            xs = xspool.tile([KDIM, FREE], BF16)
            for (p_lo, p_hi) in CHUNK_BOUNDS:
                n_idx = (p_hi - p_lo) * TOK
                xgc = gpool.tile([128, 1, 4096], BF16, tag="xg")
                nc.gpsimd.dma_gather(
                    out_ap=xgc[:, :, 0:n_idx],
                    in_ap=table[:],
                    idxs_ap=idx_t[:, p_lo * 32:p_hi * 32],
                    num_idxs=n_idx,
                    num_idxs_reg=n_idx,
                    elem_size=128,
                    transpose=True,
                    single_packet=False,
                )
                piece_eng = ([nc.sync, nc.scalar] if p_lo == 0
                             else [nc.sync])
                for k in range(7):
                    t_lo = max(0, p_lo - k)
                    t_hi = min(NPOS, p_hi - k)
                    if t_lo >= t_hi:
                        continue
                    s0 = t_lo + k - p_lo
                    piece_eng[k % len(piece_eng)].dma_start(
                        out=xs[16 * k:16 * (k + 1), TOK * t_lo:TOK * t_hi],
                        in_=xgc[0:16, 0, TOK * s0:TOK * (s0 + (t_hi - t_lo))],
                    )

            # ---- 2+3. conv rounds (static schedule) + finales
            h1 = hpool.tile([128, 16, TOK], BF16, tag="h")
            h8a = hpool.tile([128, 16, TOK], FP8, tag="h8")
            accd = {}
            accp = {}
            accp_used = set()
            n_done = 0
            h8_on_dve = {}
            for (i, t0, nt, typ, first) in CONV_SCHED:
                lhsT = wconv_t[:, 128 * i:128 * (i + 1)]
                P = convp.tile([128, 2, TOK], FP32, tag="ps")
                for rpos in range(nt):
                    t = t0 + rpos
                    nc.tensor.matmul(
                        out=P[:, rpos, :], lhsT=lhsT,
                        rhs=xs[:, TOK * t:TOK * (t + 1)],
                        start=True, stop=True,
                    )
                if first:
                    accd[i] = apool.tile([128, 2, TOK], BF16, tag=f"accd{i}", name=f"accd{i}")
                ad = accd[i]
                if typ == "Z0":
                    nc.scalar.copy(out=ad[:, 0:nt, :], in_=P[:, 0:nt, :])
                elif typ == "T0TAIL":
                    # pos t0 valid rows 0:64 (w1+w2), pos t0+1 rows 0:32 (w1)
                    nc.vector.tensor_tensor(
                        out=ad[0:64, 0:1, :], in0=ad[0:64, 0:1, :],
                        in1=P[0:64, 0:1, :], op=mybir.AluOpType.max)
                    nc.vector.tensor_tensor(
                        out=ad[0:32, 1:2, :], in0=ad[0:32, 1:2, :],
                        in1=P[0:32, 1:2, :], op=mybir.AluOpType.max)
                elif typ == "X":
                    if first:
                        nc.vector.tensor_scalar_add(
                            out=ad[:, 0:nt, :], in0=P[:, 0:nt, :], scalar1=0.0)
                    else:
                        nc.vector.tensor_tensor(
                            out=ad[:, 0:nt, :], in0=ad[:, 0:nt, :],
                            in1=P[:, 0:nt, :], op=mybir.AluOpType.max)
                else:
                    s4 = spool.tile([128, 2, TOK], BF16, tag="s4")
                    nc.scalar.copy(out=s4[:, 0:nt, :], in_=P[:, 0:nt, :])
                    if typ == "Z":
                        if first:
                            nc.vector.tensor_scalar_add(
                                out=ad[:, 0:nt, :], in0=s4[:, 0:nt, :], scalar1=0.0)
                        else:
                            nc.vector.tensor_tensor(
                                out=ad[:, 0:nt, :], in0=ad[:, 0:nt, :],
                                in1=s4[:, 0:nt, :], op=mybir.AluOpType.max)
                    else:  # W: pool folds
                        if i not in accp:
                            accp[i] = apool.tile([128, 2, TOK], BF16, tag=f"accp{i}", name=f"accp{i}")
                            nc.gpsimd.tensor_scalar_add(
                                out=accp[i][:, 0:nt, :], in0=s4[:, 0:nt, :], scalar1=0.0)
                        else:
                            ap_ = accp[i]
                            nc.gpsimd.scalar_tensor_tensor(
                                out=ap_[:, 0:nt, :], in0=s4[:, 0:nt, :], scalar=0.0,
                                in1=ap_[:, 0:nt, :],
                                op0=mybir.AluOpType.add, op1=mybir.AluOpType.max)
                        accp_used.add(i)
                # finale when tile complete
                done = (t0 + nt >= TILE_TCNT[i]) or (typ == "T0TAIL")
                if done:
                    if i in accp:
                        nc.vector.tensor_tensor(
                            out=ad[:], in0=ad[:], in1=accp[i][:],
                            op=mybir.AluOpType.max)
                    m1 = mpool.tile([128, TOK], BF16, tag="m1")
                    nc.vector.tensor_tensor(
                        out=m1[:], in0=ad[:, 0, :], in1=ad[:, 1, :],
                        op=mybir.AluOpType.max)
                    nc.vector.tensor_scalar(
                        out=h1[:, i, :], in0=m1[:],
                        scalar1=bconv_t[:, i:i + 1], scalar2=0.0,
                        op0=mybir.AluOpType.add, op1=mybir.AluOpType.max)
                    nc.gpsimd.dma_start(out=h8a[:, i, :], in_=h1[:, i, :])
                    n_done += 1

            # ---- 4. highway layers
            h_in, h8_in = h1, h8a
            for layer in range(2):
                h_out = hpool.tile([128, 16, TOK], BF16, tag="h", name=f"h_l{layer}")
                h8_out = None
                if layer == 0:
                    h8_out = hpool.tile([128, 16, TOK], FP8, tag="h8", name="h8b")
                for j in range(16):
                    wn = wpool.tile([128, 10, 128], BF16, tag="wnh", bufs=3)
                    nc.sync.dma_start(out=wn[:], in_=whw_nl[layer, j])
                    wn8 = wpool.tile([128, 3, 2, 128], FP8, tag="wn8", bufs=3)
                    nc.sync.dma_start(out=wn8[:], in_=whw_n8[layer, j])
                    wg = wpool.tile([128, 8, 2, 128], FP8, tag="wg")
                    nc.sync.dma_start(out=wg[:], in_=whw_g8[layer, j])
                    p_nl = convp.tile([128, 2, TOK], FP32, tag="ps", name="psnl")[:, 0, :]
                    corder = [c for c in (C_ORDER if layer == 0 else range(16)) if c < 10]
                    for ci, c in enumerate(corder):
                        nc.tensor.matmul(
                            out=p_nl[:], lhsT=wn[:, c, :], rhs=h_in[:, c, :],
                            start=(ci == 0), stop=False)
                    for c2 in (5, 6, 7):
                        nc.tensor.matmul(
                            out=p_nl[:], lhsT=wn8[:, c2 - 5],
                            rhs=h8_in[:, 2 * c2:2 * c2 + 2, :],
                            start=False, stop=(c2 == 7),
                            perf_mode=mybir.MatmulPerfMode.DoubleRow)
                    p_g = convp.tile([128, 2, TOK], FP32, tag="ps", name="psg")[:, 0, :]
                    g_order = C2_ORDER if layer == 0 else list(range(8))
                    for gi, c2 in enumerate(g_order):
                        nc.tensor.matmul(
                            out=p_g[:], lhsT=wg[:, c2], rhs=h8_in[:, 2 * c2:2 * c2 + 2, :],
                            start=(gi == 0), stop=(gi == 7),
                            perf_mode=mybir.MatmulPerfMode.DoubleRow)
                    nl = mpool.tile([128, TOK], BF16, tag="nl")
                    gt = mpool.tile([128, TOK], BF16, tag="gt")
                    nc.scalar.activation(
                        out=nl[:], in_=p_nl[:],
                        func=mybir.ActivationFunctionType.Relu,
                        bias=bhw_t[:, layer, j, 0:1], scale=1.0)
                    nc.scalar.activation(
                        out=gt[:], in_=p_g[:],
                        func=mybir.ActivationFunctionType.Sigmoid,
                        bias=bhw_t[:, layer, j, 1:2], scale=1.0)
                    d = mpool.tile([128, TOK], BF16, tag="d")
                    nc.vector.tensor_tensor(
                        out=d[:], in0=h_in[:, j, :], in1=nl[:],
                        op=mybir.AluOpType.subtract)
                    nc.vector.tensor_mul(out=gt[:], in0=gt[:], in1=d[:])
                    nc.vector.tensor_add(out=h_out[:, j, :], in0=nl[:], in1=gt[:])
                    if h8_out is not None:
                        if j >= 13:
                            nc.vector.tensor_scalar_add(
                                out=h8_out[:, j, :], in0=h_out[:, j, :], scalar1=0.0)
                        else:
                            nc.gpsimd.dma_start(out=h8_out[:, j, :], in_=h_out[:, j, :])
                h_in = h_out
                if h8_out is not None:
                    h8_in = h8_out

            # ---- 5. projection, feature-major out
            for j2 in range(4):
                wp = wpool.tile([128, 16, 128], BF16, tag="wn", bufs=3)
                nc.sync.dma_start(out=wp[:], in_=wproj[j2])
                p_o = convp.tile([128, 2, TOK], FP32, tag="ps", name="pso")[:, 0, :]
                for c in range(16):
                    nc.tensor.matmul(
                        out=p_o[:], lhsT=wp[:, c, :], rhs=h_in[:, c, :],
                        start=(c == 0), stop=(c == 15))
                ot = cpool.tile([128, TOK], FP32, tag="ot", name="ot", bufs=2)
                nc.scalar.activation(
                    out=ot[:], in_=p_o[:],
                    func=mybir.ActivationFunctionType.Identity,
                    bias=bproj_t[:, j2:j2 + 1], scale=1.0)
                nc.sync.dma_start(out=out[:, j2, :], in_=ot[:])

    nc.compile()
    return nc


_CACHED = {}


def _prep(inputs):
    """Host-side layout prep: sharding, index packing, weight packing."""
    chars = np.asarray(inputs["chars"]).astype(np.int64).reshape(NTOK, L)
    chars_pad = np.full((NTOK, TP), ZERO_ROW, np.int64)
    chars_pad[:, :L] = chars

    emb = np.asarray(inputs["char_emb"], np.float32)
    table = np.zeros((PAD_V, 128), np.float32)
    table[:CHAR_VOCAB, :CHAR_DIM] = emb
    table = table.astype(ml_dtypes.bfloat16)

    wc = np.zeros((7, CHAR_DIM, N_FILTERS), np.float32)
    off = 0
    for fi, (w, n) in enumerate(FILTERS):
        cw = np.asarray(inputs[f"conv_w_{fi}"], np.float32)  # (n, 16, w)
        wc[:w, :, off:off + n] = cw.transpose(2, 1, 0)
        off += n
    wconv = wc.reshape(KDIM, N_FILTERS).astype(ml_dtypes.bfloat16)
    bconv = np.concatenate([np.asarray(inputs[f"conv_b_{i}"], np.float32)
                            for i in range(7)])
    bconv_dev = bconv.reshape(16, 128).T.copy()  # (128, 16)

    whw_nl = np.zeros((2, 16, 128, 10, 128), np.float32)
    whw_n8 = np.zeros((2, 16, 128, 3, 2, 128), np.float32)
    whw_g8 = np.zeros((2, 16, 128, 8, 2, 128), np.float32)
    bhw = np.zeros((128, 2, 16, 2), np.float32)
    for l in range(2):
        W = np.asarray(inputs[f"hw_w_{l}"], np.float32)   # (4096, 2048)
        bb = np.asarray(inputs[f"hw_b_{l}"], np.float32)  # (4096,)
        WT = W.T  # (ic 2048, oc 4096)
        # nl: (j, p, c, o) = WT[128c+p, 128j+o]; chunks 12-15 go fp8 DR
        nlv = WT[:, 0:2048].reshape(16, 128, 16, 128)       # (c, p, j, o)
        whw_nl[l] = nlv[0:10].transpose(2, 1, 0, 3)         # (j, p, c, o)
        n8v = nlv[10:16].reshape(3, 2, 128, 16, 128)        # (c2, i, p, j, o)
        whw_n8[l] = n8v.transpose(3, 2, 0, 1, 4)            # (j, p, c2, i, o)
        gv = WT[:, 2048:4096].reshape(8, 2, 128, 16, 128)   # (c2, i, p, j, o)
        whw_g8[l] = gv.transpose(3, 2, 0, 1, 4)             # (j, p, c2, i, o)
        bhw[:, l, :, 0] = bb[0:2048].reshape(16, 128).T
        bhw[:, l, :, 1] = bb[2048:4096].reshape(16, 128).T
    whw_nl = whw_nl.astype(ml_dtypes.bfloat16)
    whw_n8 = whw_n8.astype(ml_dtypes.float8_e4m3)
    whw_g8 = whw_g8.astype(ml_dtypes.float8_e4m3)

    Wp = np.asarray(inputs["proj_w"], np.float32)  # (512, 2048)
    WpT = Wp.T  # (2048, 512)
    wproj = WpT.reshape(16, 128, 4, 128).transpose(2, 1, 0, 3).copy()
    wproj = wproj.astype(ml_dtypes.bfloat16)      # (j2, p, c, o)
    bproj = np.asarray(inputs["proj_b"], np.float32).reshape(4, 128).T.copy()

    shared = dict(table=table, wconv=wconv, bconv=bconv_dev, whw_nl=whw_nl, whw_n8=whw_n8,
                  whw_g8=whw_g8, bhw=bhw, wproj=wproj, bproj=bproj)

    in_maps = []
    for core in range(N_CORES):
        cp = chars_pad[core * TOK:(core + 1) * TOK]  # (512, 56)
        idx_flat = cp.T.reshape(-1).astype(np.int16)  # j = t'*512 + n
        idx16 = idx_flat.reshape(NI // 16, 16).T.copy()  # (16, NI/16)
        idx16 = np.tile(idx16, (8, 1))  # (128, NI/16)
        m = dict(shared)
        m["idx"] = idx16
        in_maps.append(m)
    return in_maps


def kernel(**inputs) -> np.ndarray:
    if "nc" not in _CACHED:
        _CACHED["nc"] = build_module()
    nc = _CACHED["nc"]
    in_maps = _prep(inputs)
    res = run_bass_kernel_spmd(nc, in_maps, core_ids=list(range(N_CORES)))
    # out[p, j2, n] -> full[n, 128*j2 + p]
    parts = []
    for r in res.results:
        o = r["out"]  # (128, 4, 512)
        parts.append(o.transpose(2, 1, 0).reshape(TOK, PROJ_DIM))
    full = np.concatenate(parts, axis=0)
    return full.reshape(B, S, PROJ_DIM)


if __name__ == "__main__":
    order, load = CONV_SCHED, CONV_LOAD
    from collections import Counter
    print("rounds:", len(order), Counter(t for (_, _, _, t, _) in order))
    print("loads (us):", {k: round(v / 1000, 1) for k, v in load.items()})
